# revision 23
# baseline (speedup 1.0000x reference)
"""CQAttention (context-query attention) Bass kernel for 8 NeuronCores.

Full inputs:  C [64,128,1000] f32, Q [64,128,100] f32, W [64000,1,384] f32
Full output:  [64, 512, 1000] f32

Sharding: pure data-parallel on the batch dim - 8 batches per core.

Per-batch math (D=128, Lc=1000, Lq=100):
  Ct = C.T [Lc,D], Qt = Q.T [Lq,D], w1/w2/w3 = W row blocks [Lc,D]
  U  = w1 + w3*Ct ; v = rowsum(w2*Ct)
  S  = U @ Q + v  (the v term drops out of the row softmax S1)
  S1 = softmax_cols(S) ; S2 = softmax_rows(S)
  A  = S1 @ Qt ; Bm = S1 @ (S2^T @ Ct)
  out = concat([Ct, A, Ct*A, Ct*Bm], 1).T  -> [4D, Lc]

Layout notes:
 - Lc is tiled 8 x 125 with the INTERLEAVED mapping i = p*8 + t (p =
   partition, t = tile) so the W DMA reads 12KB contiguous per partition.
   All intermediate tensors with an Lc axis are kept in the permuted
   (t-major) order; the final output ops unpermute via strided APs.
 - Scores are built transposed (S0T [Lq, Lc]) so the big matmuls run with
   float32r operands at full PE rate (moving free dim >= 256).
 - S1 normalization: column sums of exp(S0T) via a ones-vector matmul,
   reciprocal, then a K=1 matmul broadcast across partitions.
 - S2 path: PE-transpose exp(S0T) tiles, scale by exp(v), cast bf16, and
   contract with bf16 Ct tiles (ones column appended for the s2 sums).
"""

import numpy as np

B, D, LC, LQ = 64, 128, 1000, 100
NCORES = 8
NB = B // NCORES   # batches per core
NT = 8             # LC tiles
TL = LC // NT      # 125

_cache = {}


def _build(tu_bf16=True, v_bf16=True):
    import concourse.bass as bass
    import concourse.tile as tile
    from concourse import bacc, mybir, masks
    from contextlib import ExitStack

    f32 = mybir.dt.float32
    f32r = mybir.dt.float32r
    bf16 = mybir.dt.bfloat16
    AF = mybir.ActivationFunctionType
    ALU = mybir.AluOpType
    AX = mybir.AxisListType

    ct_dt = bf16 if tu_bf16 else f32
    e2_dt = bf16 if tu_bf16 else f32

    nc = bacc.Bacc("TRN2", target_bir_lowering=False, debug=False,
                   num_devices=NCORES)
    C_d = nc.dram_tensor("C", [NB, D, LC], f32, kind="ExternalInput").ap()
    Q_d = nc.dram_tensor("Q", [NB, D, LQ], f32, kind="ExternalInput").ap()
    W_d = nc.dram_tensor("W", [NB, LC, 3 * D], f32, kind="ExternalInput").ap()
    O_d = nc.dram_tensor("OUT", [NB, 4 * D, LC], f32, kind="ExternalOutput").ap()

    with tile.TileContext(nc) as tc, ExitStack() as ctx:
        const_pool = ctx.enter_context(tc.tile_pool(name="const", bufs=1))
        ident = const_pool.tile([128, 128], f32)
        masks.make_identity(nc, ident[:])
        identr = const_pool.tile([128, 128], f32r)
        nc.scalar.activation(identr[:], ident[:], AF.Copy)
        # f32r ones for the s1 column-sum / broadcast matmuls
        ones_f = const_pool.tile([128, 1], f32)
        nc.vector.memset(ones_f[:], 1.0)
        ones_col = const_pool.tile([128, 1], f32r)
        nc.scalar.activation(ones_col[:], ones_f[:], AF.Copy)
        ones_rf = const_pool.tile([1, 128], f32)
        nc.vector.memset(ones_rf[:], 1.0)
        ones_row = const_pool.tile([1, 128], f32r)
        nc.scalar.activation(ones_row[:], ones_rf[:], AF.Copy)
        zero_f = const_pool.tile([128, 1], f32)
        nc.vector.memset(zero_f[:], 0.0)

        sb = ctx.enter_context(tc.tile_pool(name="sb", bufs=2))
        small = ctx.enter_context(tc.tile_pool(name="small", bufs=3))
        outp = ctx.enter_context(tc.tile_pool(name="outp", bufs=2))
        tp_ps = ctx.enter_context(tc.tile_pool(name="tp_ps", bufs=3, space="PSUM"))
        mm_ps = ctx.enter_context(tc.tile_pool(name="mm_ps", bufs=5, space="PSUM"))

        for b in range(NB):
            # ---- loads ----
            # w_sb[p, t, c] = W[b, p*8+t, c]  (12KB contiguous per partition)
            w_sb = sb.tile([TL, NT * 3 * D], f32, tag="w", name=f"w{b}")
            nc.gpsimd.dma_start(
                w_sb[:].rearrange("p (t c) -> p t c", c=3 * D),
                W_d[b].rearrange("(p t) c -> p t c", t=NT))
            c_sb = sb.tile([D, LC], f32, tag="c", name=f"c{b}")
            nc.sync.dma_start(c_sb[:], C_d[b])
            q_sb = sb.tile([D, LQ], f32, tag="q", name=f"q{b}")
            nc.sync.dma_start(q_sb[:], Q_d[b])

            # ---- Qt (early: only needs the Q load) ----
            qtp = tp_ps.tile([LQ, D], f32, tag="tp", name=f"qtp{b}")
            nc.tensor.transpose(qtp[:], q_sb[:], ident[:])
            qt_sb = small.tile([LQ, D], f32r, tag="qt", name=f"qt{b}")
            nc.vector.tensor_copy(qt_sb[:], qtp[:])
            q_r = small.tile([D, LQ], f32r, tag="q_r", name=f"q_r{b}")
            nc.vector.tensor_copy(q_r[:], q_sb[:])

            # views with the interleaved Lc mapping  i = p*8 + t
            c_r = sb.tile([D, LC], f32r, tag="c_r", name=f"c_r{b}")
            nc.vector.tensor_copy(c_r[:], c_sb[:])
            c_tiles = c_r[:].rearrange("d (p t) -> d t p", t=NT)  # [D, t, p]
            wv = w_sb[:].rearrange("p (t c) -> p t c", c=3 * D)
            w1 = wv[:, :, 0:D]
            w2 = wv[:, :, D:2 * D]
            w3 = wv[:, :, 2 * D:3 * D]

            # ---- Ct tiles: PE transpose groups of 4; keep PSUM f32 copy
            #      for U/v, write bf16 (+ones col) SBUF copy for Tu ----
            ct_sb = sb.tile([TL, NT * (D + 1)], ct_dt, tag="ct", name=f"ct{b}")
            ctv = ct_sb[:].rearrange("p (t c) -> p t c", c=D + 1)
            nc.vector.memset(ctv[:, :, D:D + 1], 1.0)
            w3ct = sb.tile([TL, NT * D], f32, tag="w3ct", name=f"w3ct{b}")
            w3ctv = w3ct[:].rearrange("p (t c) -> p t c", c=D)
            vtmp = sb.tile([TL, NT * D], f32, tag="vtmp", name=f"vtmp{b}")
            vtmpv = vtmp[:].rearrange("p (t c) -> p t c", c=D)
            for g in range(2):
                ctp = tp_ps.tile([TL, 4 * D], f32r, tag="tp", name=f"ctp{b}_{g}")
                for k in range(4):
                    t = 4 * g + k
                    nc.tensor.transpose(
                        ctp[:, k * D:(k + 1) * D],
                        c_tiles[:, t, :], identr[:])
                ctpv = ctp[:].bitcast(f32).rearrange("p (k c) -> p k c", c=D)
                gs = slice(4 * g, 4 * g + 4)
                nc.vector.tensor_copy(ctv[:, gs, 0:D], ctpv)
                nc.vector.tensor_tensor(out=w3ctv[:, gs, :], in0=w3[:, gs, :],
                                        in1=ctpv, op=ALU.mult)
                if not v_bf16:
                    nc.vector.tensor_tensor(out=vtmpv[:, gs, :],
                                            in0=w2[:, gs, :], in1=ctpv,
                                            op=ALU.mult)
            if v_bf16:
                # gpsimd is idle; feed it the v multiply from the bf16 ct
                nc.gpsimd.tensor_tensor(out=vtmpv, in0=w2,
                                        in1=ctv[:, :, 0:D], op=ALU.mult)
            v_all = small.tile([TL, NT], f32, tag="v", name=f"v{b}")
            nc.vector.tensor_reduce(v_all[:], vtmpv, axis=AX.X, op=ALU.add)
            expv = small.tile([TL, NT], f32, tag="expv", name=f"expv{b}")
            nc.scalar.activation(expv[:], v_all[:], AF.Exp)

            # ---- U^T via PSUM-accumulating PE transposes:
            #      utp = w1_tile^T  (+)  (w3*Ct)_tile^T  ----
            ut_sb = sb.tile([D, 8 * D], f32r, tag="ut", name=f"ut{b}")
            utv = ut_sb[:].rearrange("d (t c) -> d t c", c=D)
            nc.vector.tensor_copy(
                utv[:, :, TL:D],
                zero_f[:, 0:1].to_broadcast((D, NT, D - TL)))
            e1t_sb = sb.tile([LQ, 8 * D], f32r, tag="e1t", name=f"e1t{b}")
            s0h = []
            u_all = sb.tile([D, NT * D], f32r, tag="u", name=f"u{b}")
            nc.vector.tensor_copy(
                u_all[96:D, :].rearrange("p (t c) -> p t c", c=D),
                zero_f[96:D, 0:1].to_broadcast((D - 96, NT, D)))
            nc.vector.tensor_tensor(
                out=u_all[0:TL, :].rearrange("p (t c) -> p t c", c=D),
                in0=w3ctv, in1=w1, op=ALU.add)
            for g in range(2):
                utp = tp_ps.tile([D, 4 * D], f32r, tag="tp", name=f"utp{b}_{g}")
                for k in range(4):
                    t = 4 * g + k
                    nc.tensor.transpose(utp[:, k * D:(k + 1) * D],
                                        u_all[:, t * D:(t + 1) * D],
                                        identr[:])
                nc.vector.tensor_copy(
                    utv[:, 4 * g:4 * g + 4, 0:TL],
                    utp[:].rearrange("d (k c) -> d k c", c=D)[:, :, 0:TL])
                s0g = mm_ps.tile([LQ, 512], f32, tag="mmh", name=f"s0t{b}_{g}")
                s0h.append(s0g)
                nc.tensor.matmul(s0g[:], q_r[:],
                                 ut_sb[:, g * 512:(g + 1) * 512],
                                 start=True, stop=True)
                nc.scalar.activation(e1t_sb[:, g * 512:(g + 1) * 512],
                                     s0g[:], AF.Exp)

            # ---- E2 tiles = transpose(E1T) * exp(v); row-sums of E1 tiles
            #      feed the s1 normalization ----
            e2_all = sb.tile([TL, NT * LQ], e2_dt, tag="e2", name=f"e2{b}")
            e2v = e2_all[:].rearrange("p (t c) -> p t c", c=LQ)
            s1a = small.tile([TL, NT], f32, tag="s1a", name=f"s1a{b}")
            for g in range(2):
                e1p = tp_ps.tile([TL, 4 * LQ], f32r, tag="tp",
                                 name=f"e1p{b}_{g}")
                for k in range(4):
                    t = 4 * g + k
                    nc.tensor.transpose(
                        e1p[:, k * LQ:(k + 1) * LQ],
                        e1t_sb[:, t * D:t * D + TL],
                        identr[0:LQ, 0:LQ])
                e1pv = e1p[:].rearrange("p (k c) -> p k c", c=LQ)
                scl = expv[:, 4 * g:4 * g + 4].unsqueeze(-1).to_broadcast(
                    (TL, 4, LQ))
                nc.vector.tensor_tensor(
                    out=e2v[:, 4 * g:4 * g + 4, :], in0=e1pv,
                    in1=scl, op=ALU.mult)
                nc.vector.tensor_reduce(s1a[:, 4 * g:4 * g + 4], e1pv,
                                        axis=AX.X, op=ALU.add)

            # ---- 1/s1 -> [1, 1024] row -> partition broadcast ----
            s1ra = small.tile([D, NT], f32, tag="s1ra", name=f"s1ra{b}")
            nc.vector.memset(s1ra[96:D, :], 1.0)
            nc.vector.reciprocal(s1ra[0:TL, :], s1a[:])
            s1rp = tp_ps.tile([NT, D], f32, tag="tp", name=f"s1rp{b}")
            nc.tensor.transpose(s1rp[:], s1ra[:], ident[:])
            s1st = small.tile([NT, D], f32r, tag="s1st", name=f"s1st{b}")
            nc.vector.tensor_copy(s1st[:], s1rp[:])
            s1row = small.tile([1, 8 * D], f32r, tag="s1row", name=f"s1row{b}")
            nc.sync.dma_start(s1row[:], s1st[:])
            s1bb = sb.tile([LQ, 8 * D], f32r, tag="s1bb", name=f"s1bb{b}")
            nc.gpsimd.partition_broadcast(s1bb[:], s1row[:], channels=LQ)
            s1t = sb.tile([LQ, 8 * D], f32r, tag="s1t", name=f"s1t{b}")
            nc.vector.tensor_tensor(out=s1t[:], in0=e1t_sb[:], in1=s1bb[:],
                                    op=ALU.mult)

            # ---- Tu = E2^T @ [Ct | 1]  (accumulate over tiles) ----
            tu = tp_ps.tile([LQ, D + 1], f32, tag="tp", name=f"tu{b}")
            for t in range(NT):
                nc.tensor.matmul(tu[:], e2v[:, t, :], ctv[:, t, :],
                                 start=(t == 0), stop=(t == NT - 1))
            s2r = small.tile([LQ, 1], f32, tag="s2r", name=f"s2r{b}")
            nc.vector.reciprocal(s2r[:], tu[:, D:D + 1])
            that_sb = small.tile([LQ, D], f32r, tag="that", name=f"that{b}")
            nc.vector.tensor_scalar_mul(that_sb[:], tu[:, 0:D], s2r[:])

            # ---- A^T and Bm^T (per half) + unpermuted outputs ----
            cpt = c_sb[:].rearrange("d (p t) -> d p t", t=NT)
            oa = outp.tile([D, LC], f32, tag="oa", name=f"oa{b}")
            oca = outp.tile([D, LC], f32, tag="oca", name=f"oca{b}")
            ocb = outp.tile([D, LC], f32, tag="ocb", name=f"ocb{b}")
            nc.sync.dma_start(O_d[b, 0:D], c_sb[:])
            for g in range(2):
                gsl = slice(g * 512, (g + 1) * 512)
                tsl = slice(4 * g, 4 * g + 4)
                ath = mm_ps.tile([D, 512], f32, tag="mmh", name=f"at{b}_{g}")
                nc.tensor.matmul(ath[:], qt_sb[:], s1t[:, gsl],
                                 start=True, stop=True)
                bmh = mm_ps.tile([D, 512], f32, tag="mmh", name=f"bm{b}_{g}")
                nc.tensor.matmul(bmh[:], that_sb[:], s1t[:, gsl],
                                 start=True, stop=True)
                athp = ath[:].rearrange("d (t c) -> d c t", c=D)[:, 0:TL, :]
                bmhp = bmh[:].rearrange("d (t c) -> d c t", c=D)[:, 0:TL, :]
                oav = oa[:].rearrange("d (p t) -> d p t", t=NT)[:, :, tsl]
                ocav = oca[:].rearrange("d (p t) -> d p t", t=NT)[:, :, tsl]
                ocbv = ocb[:].rearrange("d (p t) -> d p t", t=NT)[:, :, tsl]
                cpg = cpt[:, :, tsl]
                nc.scalar.activation(oav, athp, AF.Copy)
                nc.vector.tensor_tensor(out=ocav, in0=cpg, in1=athp,
                                        op=ALU.mult)
                nc.vector.tensor_tensor(out=ocbv, in0=cpg, in1=bmhp,
                                        op=ALU.mult)
            nc.sync.dma_start(O_d[b, D:2 * D], oa[:])
            nc.sync.dma_start(O_d[b, 2 * D:3 * D], oca[:])
            nc.sync.dma_start(O_d[b, 3 * D:4 * D], ocb[:])

    nc.compile()
    return nc


def _get_nc(**kw):
    key = tuple(sorted(kw.items()))
    if key not in _cache:
        _cache[key] = _build(**kw)
    return _cache[key]


def kernel(C, Q, W, **build_kw):
    from concourse import bass_utils

    C = np.ascontiguousarray(C, np.float32)
    Q = np.ascontiguousarray(Q, np.float32)
    Wr = np.ascontiguousarray(W, np.float32).reshape(NCORES, NB, LC, 3 * D)
    Cs = C.reshape(NCORES, NB, D, LC)
    Qs = Q.reshape(NCORES, NB, D, LQ)

    nc = _get_nc(**build_kw)
    in_maps = [{"C": Cs[i], "Q": Qs[i], "W": Wr[i]} for i in range(NCORES)]
    res = bass_utils.run_bass_kernel_spmd(nc, in_maps,
                                          core_ids=list(range(NCORES)))
    out = np.concatenate([res.results[i]["OUT"] for i in range(NCORES)], 0)
    return out.astype(np.float32)


# revision 24
# speedup vs baseline: 1.1889x; 1.1889x over previous
"""CQAttention (context-query attention) Bass kernel for 8 NeuronCores.

Full inputs:  C [64,128,1000] f32, Q [64,128,100] f32, W [64000,1,384] f32
Full output:  [64, 512, 1000] f32

Sharding: pure data-parallel on the batch dim - 8 batches per core.

Per-batch math (D=128, Lc=1000, Lq=100):
  Ct = C.T [Lc,D], Qt = Q.T [Lq,D], w1/w2/w3 = W row blocks [Lc,D]
  U  = w1 + w3*Ct ; v = rowsum(w2*Ct)
  S  = U @ Q + v  (the v term drops out of the row softmax S1)
  S1 = softmax_cols(S) ; S2 = softmax_rows(S)
  A  = S1 @ Qt ; Bm = S1 @ (S2^T @ Ct)
  out = concat([Ct, A, Ct*A, Ct*Bm], 1).T  -> [4D, Lc]

Layout notes:
 - Lc is tiled 8 x 125 with the INTERLEAVED mapping i = p*8 + t (p =
   partition, t = tile) so the W DMA reads 12KB contiguous per partition.
   All intermediate tensors with an Lc axis are kept in the permuted
   (t-major) order; the final output ops unpermute via strided APs.
 - Scores are built transposed (S0T [Lq, Lc]) so the big matmuls run with
   float32r operands at full PE rate (moving free dim >= 256).
 - S1 normalization: column sums of exp(S0T) via a ones-vector matmul,
   reciprocal, then a K=1 matmul broadcast across partitions.
 - S2 path: PE-transpose exp(S0T) tiles, scale by exp(v), cast bf16, and
   contract with bf16 Ct tiles (ones column appended for the s2 sums).
"""

import numpy as np

B, D, LC, LQ = 64, 128, 1000, 100
NCORES = 8
NB = B // NCORES   # batches per core
NT = 8             # LC tiles
TL = LC // NT      # 125

_cache = {}


def _build(tu_bf16=True, v_bf16=True):
    import concourse.bass as bass
    import concourse.tile as tile
    from concourse import bacc, mybir, masks
    from contextlib import ExitStack

    f32 = mybir.dt.float32
    f32r = mybir.dt.float32r
    bf16 = mybir.dt.bfloat16
    AF = mybir.ActivationFunctionType
    ALU = mybir.AluOpType
    AX = mybir.AxisListType

    ct_dt = bf16 if tu_bf16 else f32
    e2_dt = bf16 if tu_bf16 else f32

    nc = bacc.Bacc("TRN2", target_bir_lowering=False, debug=False,
                   num_devices=NCORES)
    C_d = nc.dram_tensor("C", [NB, D, LC], f32, kind="ExternalInput").ap()
    Q_d = nc.dram_tensor("Q", [NB, D, LQ], f32, kind="ExternalInput").ap()
    W_d = nc.dram_tensor("W", [NB, LC, 3 * D], f32, kind="ExternalInput").ap()
    O_d = nc.dram_tensor("OUT", [NB, 4 * D, LC], f32, kind="ExternalOutput").ap()

    with tile.TileContext(nc) as tc, ExitStack() as ctx:
        const_pool = ctx.enter_context(tc.tile_pool(name="const", bufs=1))
        ident = const_pool.tile([128, 128], f32)
        masks.make_identity(nc, ident[:])
        identr = const_pool.tile([128, 128], f32r)
        nc.scalar.activation(identr[:], ident[:], AF.Copy)
        # f32r ones for the s1 column-sum / broadcast matmuls
        ones_f = const_pool.tile([128, 1], f32)
        nc.vector.memset(ones_f[:], 1.0)
        ones_col = const_pool.tile([128, 1], f32r)
        nc.scalar.activation(ones_col[:], ones_f[:], AF.Copy)
        ones_rf = const_pool.tile([1, 128], f32)
        nc.vector.memset(ones_rf[:], 1.0)
        ones_row = const_pool.tile([1, 128], f32r)
        nc.scalar.activation(ones_row[:], ones_rf[:], AF.Copy)
        zero_f = const_pool.tile([128, 1], f32)
        nc.vector.memset(zero_f[:], 0.0)

        sb = ctx.enter_context(tc.tile_pool(name="sb", bufs=2))
        small = ctx.enter_context(tc.tile_pool(name="small", bufs=3))
        outp = ctx.enter_context(tc.tile_pool(name="outp", bufs=2))
        tp_ps = ctx.enter_context(tc.tile_pool(name="tp_ps", bufs=3, space="PSUM"))
        mm_ps = ctx.enter_context(tc.tile_pool(name="mm_ps", bufs=5, space="PSUM"))

        for b in range(NB):
            # ---- loads ----
            # w_sb[p, t, c] = W[b, p*8+t, c]  (12KB contiguous per partition)
            w_sb = sb.tile([TL, NT * 3 * D], f32, tag="w", name=f"w{b}", bufs=3)
            nc.gpsimd.dma_start(
                w_sb[:].rearrange("p (t c) -> p t c", c=3 * D),
                W_d[b].rearrange("(p t) c -> p t c", t=NT))
            c_sb = sb.tile([D, LC], f32, tag="c", name=f"c{b}", bufs=3)
            nc.sync.dma_start(c_sb[:], C_d[b])
            q_sb = sb.tile([D, LQ], f32, tag="q", name=f"q{b}", bufs=3)
            nc.sync.dma_start(q_sb[:], Q_d[b])

            # ---- Qt (early: only needs the Q load) ----
            qtp = tp_ps.tile([LQ, D], f32, tag="tp", name=f"qtp{b}")
            nc.tensor.transpose(qtp[:], q_sb[:], ident[:])
            qt_sb = small.tile([LQ, D], f32r, tag="qt", name=f"qt{b}")
            nc.vector.tensor_copy(qt_sb[:], qtp[:])
            q_r = small.tile([D, LQ], f32r, tag="q_r", name=f"q_r{b}")
            nc.vector.tensor_copy(q_r[:], q_sb[:])

            # views with the interleaved Lc mapping  i = p*8 + t
            c_r = sb.tile([D, LC], f32r, tag="c_r", name=f"c_r{b}")
            nc.scalar.activation(c_r[:], c_sb[:], AF.Copy)
            c_tiles = c_r[:].rearrange("d (p t) -> d t p", t=NT)  # [D, t, p]
            wv = w_sb[:].rearrange("p (t c) -> p t c", c=3 * D)
            w1 = wv[:, :, 0:D]
            w2 = wv[:, :, D:2 * D]
            w3 = wv[:, :, 2 * D:3 * D]

            # ---- Ct tiles: PE transpose groups of 4; keep PSUM f32 copy
            #      for U/v, write bf16 (+ones col) SBUF copy for Tu ----
            ct_sb = sb.tile([TL, NT * (D + 1)], ct_dt, tag="ct", name=f"ct{b}")
            ctv = ct_sb[:].rearrange("p (t c) -> p t c", c=D + 1)
            nc.vector.memset(ctv[:, :, D:D + 1], 1.0)
            w3ct = sb.tile([TL, NT * D], f32, tag="w3ct", name=f"w3ct{b}")
            w3ctv = w3ct[:].rearrange("p (t c) -> p t c", c=D)
            vtmp = sb.tile([TL, NT * D], f32, tag="vtmp", name=f"vtmp{b}")
            vtmpv = vtmp[:].rearrange("p (t c) -> p t c", c=D)
            for g in range(2):
                ctp = tp_ps.tile([TL, 4 * D], f32r, tag="tp", name=f"ctp{b}_{g}")
                for k in range(4):
                    t = 4 * g + k
                    nc.tensor.transpose(
                        ctp[:, k * D:(k + 1) * D],
                        c_tiles[:, t, :], identr[:])
                ctpv = ctp[:].bitcast(f32).rearrange("p (k c) -> p k c", c=D)
                gs = slice(4 * g, 4 * g + 4)
                nc.scalar.activation(ctv[:, gs, 0:D], ctpv, AF.Copy)
                nc.vector.tensor_tensor(out=w3ctv[:, gs, :], in0=w3[:, gs, :],
                                        in1=ctpv, op=ALU.mult)
                if not v_bf16:
                    nc.vector.tensor_tensor(out=vtmpv[:, gs, :],
                                            in0=w2[:, gs, :], in1=ctpv,
                                            op=ALU.mult)
            if v_bf16:
                # gpsimd is idle; feed it the v multiply from the bf16 ct
                nc.gpsimd.tensor_tensor(out=vtmpv, in0=w2,
                                        in1=ctv[:, :, 0:D], op=ALU.mult)
            v_all = small.tile([TL, NT], f32, tag="v", name=f"v{b}")
            nc.vector.tensor_reduce(v_all[:], vtmpv, axis=AX.X, op=ALU.add)
            expv = small.tile([TL, NT], f32, tag="expv", name=f"expv{b}")
            nc.scalar.activation(expv[:], v_all[:], AF.Exp)

            # ---- U^T via PSUM-accumulating PE transposes:
            #      utp = w1_tile^T  (+)  (w3*Ct)_tile^T  ----
            ut_sb = sb.tile([D, 8 * D], f32r, tag="ut", name=f"ut{b}")
            utv = ut_sb[:].rearrange("d (t c) -> d t c", c=D)
            nc.scalar.activation(
                utv[:, :, TL:D],
                zero_f[:, 0:1].to_broadcast((D, NT, D - TL)), AF.Copy)
            e1t_sb = sb.tile([LQ, 8 * D], f32r, tag="e1t", name=f"e1t{b}")
            s0h = []
            u_all = sb.tile([D, NT * D], f32r, tag="u", name=f"u{b}")
            nc.scalar.activation(
                u_all[96:D, :].rearrange("p (t c) -> p t c", c=D),
                zero_f[96:D, 0:1].to_broadcast((D - 96, NT, D)), AF.Copy)
            nc.vector.tensor_tensor(
                out=u_all[0:TL, :].rearrange("p (t c) -> p t c", c=D),
                in0=w3ctv, in1=w1, op=ALU.add)
            for g in range(2):
                utp = tp_ps.tile([D, 4 * D], f32r, tag="tp", name=f"utp{b}_{g}")
                for k in range(4):
                    t = 4 * g + k
                    nc.tensor.transpose(utp[:, k * D:(k + 1) * D],
                                        u_all[:, t * D:(t + 1) * D],
                                        identr[:])
                nc.scalar.activation(
                    utv[:, 4 * g:4 * g + 4, 0:TL],
                    utp[:].rearrange("d (k c) -> d k c", c=D)[:, :, 0:TL],
                    AF.Copy)
                s0g = mm_ps.tile([LQ, 512], f32, tag="mmh", name=f"s0t{b}_{g}")
                s0h.append(s0g)
                nc.tensor.matmul(s0g[:], q_r[:],
                                 ut_sb[:, g * 512:(g + 1) * 512],
                                 start=True, stop=True)
                nc.scalar.activation(e1t_sb[:, g * 512:(g + 1) * 512],
                                     s0g[:], AF.Exp)

            # ---- E2 tiles = transpose(E1T) * exp(v); row-sums of E1 tiles
            #      feed the s1 normalization ----
            e2_all = sb.tile([TL, NT * LQ], e2_dt, tag="e2", name=f"e2{b}")
            e2v = e2_all[:].rearrange("p (t c) -> p t c", c=LQ)
            s1a = small.tile([TL, NT], f32, tag="s1a", name=f"s1a{b}")
            for g in range(2):
                e1p = tp_ps.tile([TL, 4 * LQ], f32r, tag="tp",
                                 name=f"e1p{b}_{g}")
                for k in range(4):
                    t = 4 * g + k
                    nc.tensor.transpose(
                        e1p[:, k * LQ:(k + 1) * LQ],
                        e1t_sb[:, t * D:t * D + TL],
                        identr[0:LQ, 0:LQ])
                e1pv = e1p[:].rearrange("p (k c) -> p k c", c=LQ)
                scl = expv[:, 4 * g:4 * g + 4].unsqueeze(-1).to_broadcast(
                    (TL, 4, LQ))
                nc.vector.tensor_tensor(
                    out=e2v[:, 4 * g:4 * g + 4, :], in0=e1pv,
                    in1=scl, op=ALU.mult)
                nc.vector.tensor_reduce(s1a[:, 4 * g:4 * g + 4], e1pv,
                                        axis=AX.X, op=ALU.add)

            # ---- 1/s1 -> [1, 1024] row -> partition broadcast ----
            s1ra = small.tile([D, NT], f32, tag="s1ra", name=f"s1ra{b}")
            nc.vector.memset(s1ra[96:D, :], 1.0)
            nc.vector.reciprocal(s1ra[0:TL, :], s1a[:])
            s1rp = tp_ps.tile([NT, D], f32, tag="tp", name=f"s1rp{b}")
            nc.tensor.transpose(s1rp[:], s1ra[:], ident[:])
            s1st = small.tile([NT, D], f32r, tag="s1st", name=f"s1st{b}")
            nc.vector.tensor_copy(s1st[:], s1rp[:])
            s1row = small.tile([1, 8 * D], f32r, tag="s1row", name=f"s1row{b}")
            nc.sync.dma_start(s1row[:], s1st[:])
            s1bb = sb.tile([LQ, 8 * D], f32r, tag="s1bb", name=f"s1bb{b}")
            nc.gpsimd.partition_broadcast(s1bb[:], s1row[:], channels=LQ)
            s1t = sb.tile([LQ, 8 * D], f32r, tag="s1t", name=f"s1t{b}")
            nc.vector.tensor_tensor(out=s1t[:], in0=e1t_sb[:], in1=s1bb[:],
                                    op=ALU.mult)

            # ---- Tu = E2^T @ [Ct | 1]  (accumulate over tiles) ----
            tu = tp_ps.tile([LQ, D + 1], f32, tag="tp", name=f"tu{b}")
            for t in range(NT):
                nc.tensor.matmul(tu[:], e2v[:, t, :], ctv[:, t, :],
                                 start=(t == 0), stop=(t == NT - 1))
            s2r = small.tile([LQ, 1], f32, tag="s2r", name=f"s2r{b}")
            nc.vector.reciprocal(s2r[:], tu[:, D:D + 1])
            that_sb = small.tile([LQ, D], f32r, tag="that", name=f"that{b}")
            nc.vector.tensor_scalar_mul(that_sb[:], tu[:, 0:D], s2r[:])

            # ---- A^T and Bm^T (per half) + unpermuted outputs ----
            cpt = c_sb[:].rearrange("d (p t) -> d p t", t=NT)
            oa = outp.tile([D, LC], f32, tag="oa", name=f"oa{b}")
            oca = outp.tile([D, LC], f32, tag="oca", name=f"oca{b}")
            ocb = outp.tile([D, LC], f32, tag="ocb", name=f"ocb{b}")
            nc.sync.dma_start(O_d[b, 0:D], c_sb[:])
            for g in range(2):
                gsl = slice(g * 512, (g + 1) * 512)
                tsl = slice(4 * g, 4 * g + 4)
                ath = mm_ps.tile([D, 512], f32, tag="mmh", name=f"at{b}_{g}")
                nc.tensor.matmul(ath[:], qt_sb[:], s1t[:, gsl],
                                 start=True, stop=True)
                bmh = mm_ps.tile([D, 512], f32, tag="mmh", name=f"bm{b}_{g}")
                nc.tensor.matmul(bmh[:], that_sb[:], s1t[:, gsl],
                                 start=True, stop=True)
                athp = ath[:].rearrange("d (t c) -> d c t", c=D)[:, 0:TL, :]
                bmhp = bmh[:].rearrange("d (t c) -> d c t", c=D)[:, 0:TL, :]
                oav = oa[:].rearrange("d (p t) -> d p t", t=NT)[:, :, tsl]
                ocav = oca[:].rearrange("d (p t) -> d p t", t=NT)[:, :, tsl]
                ocbv = ocb[:].rearrange("d (p t) -> d p t", t=NT)[:, :, tsl]
                cpg = cpt[:, :, tsl]
                nc.scalar.activation(oav, athp, AF.Copy)
                nc.vector.tensor_tensor(out=ocav, in0=cpg, in1=athp,
                                        op=ALU.mult)
                nc.vector.tensor_tensor(out=ocbv, in0=cpg, in1=bmhp,
                                        op=ALU.mult)
            nc.sync.dma_start(O_d[b, D:2 * D], oa[:])
            nc.sync.dma_start(O_d[b, 2 * D:3 * D], oca[:])
            nc.sync.dma_start(O_d[b, 3 * D:4 * D], ocb[:])

    nc.compile()
    return nc


def _get_nc(**kw):
    key = tuple(sorted(kw.items()))
    if key not in _cache:
        _cache[key] = _build(**kw)
    return _cache[key]


def kernel(C, Q, W, **build_kw):
    from concourse import bass_utils

    C = np.ascontiguousarray(C, np.float32)
    Q = np.ascontiguousarray(Q, np.float32)
    Wr = np.ascontiguousarray(W, np.float32).reshape(NCORES, NB, LC, 3 * D)
    Cs = C.reshape(NCORES, NB, D, LC)
    Qs = Q.reshape(NCORES, NB, D, LQ)

    nc = _get_nc(**build_kw)
    in_maps = [{"C": Cs[i], "Q": Qs[i], "W": Wr[i]} for i in range(NCORES)]
    res = bass_utils.run_bass_kernel_spmd(nc, in_maps,
                                          core_ids=list(range(NCORES)))
    out = np.concatenate([res.results[i]["OUT"] for i in range(NCORES)], 0)
    return out.astype(np.float32)


# revision 26
# speedup vs baseline: 1.2222x; 1.0280x over previous
"""CQAttention (context-query attention) Bass kernel for 8 NeuronCores.

Full inputs:  C [64,128,1000] f32, Q [64,128,100] f32, W [64000,1,384] f32
Full output:  [64, 512, 1000] f32

Sharding: pure data-parallel on the batch dim - 8 batches per core.

Per-batch math (D=128, Lc=1000, Lq=100):
  Ct = C.T [Lc,D], Qt = Q.T [Lq,D], w1/w2/w3 = W row blocks [Lc,D]
  U  = w1 + w3*Ct ; v = rowsum(w2*Ct)
  S  = U @ Q + v  (the v term drops out of the row softmax S1)
  S1 = softmax_cols(S) ; S2 = softmax_rows(S)
  A  = S1 @ Qt ; Bm = S1 @ (S2^T @ Ct)
  out = concat([Ct, A, Ct*A, Ct*Bm], 1).T  -> [4D, Lc]

Layout notes:
 - Lc is tiled 8 x 125 with the INTERLEAVED mapping i = p*8 + t (p =
   partition, t = tile) so the W DMA reads 12KB contiguous per partition.
   All intermediate tensors with an Lc axis are kept in the permuted
   (t-major) order; the final output ops unpermute via strided APs.
 - Scores are built transposed (S0T [Lq, Lc]) so the big matmuls run with
   float32r operands at full PE rate (moving free dim >= 256).
 - S1 normalization: column sums of exp(S0T) via a ones-vector matmul,
   reciprocal, then a K=1 matmul broadcast across partitions.
 - S2 path: PE-transpose exp(S0T) tiles, scale by exp(v), cast bf16, and
   contract with bf16 Ct tiles (ones column appended for the s2 sums).
"""

import numpy as np

B, D, LC, LQ = 64, 128, 1000, 100
NCORES = 8
NB = B // NCORES   # batches per core
NT = 8             # LC tiles
TL = LC // NT      # 125

_cache = {}


def _build(tu_bf16=True, v_bf16=True):
    import concourse.bass as bass
    import concourse.tile as tile
    from concourse import bacc, mybir, masks
    from contextlib import ExitStack

    f32 = mybir.dt.float32
    f32r = mybir.dt.float32r
    bf16 = mybir.dt.bfloat16
    AF = mybir.ActivationFunctionType
    ALU = mybir.AluOpType
    AX = mybir.AxisListType

    ct_dt = bf16 if tu_bf16 else f32
    e2_dt = bf16 if tu_bf16 else f32

    nc = bacc.Bacc("TRN2", target_bir_lowering=False, debug=False,
                   num_devices=NCORES)
    C_d = nc.dram_tensor("C", [NB, D, LC], f32, kind="ExternalInput").ap()
    Q_d = nc.dram_tensor("Q", [NB, D, LQ], f32, kind="ExternalInput").ap()
    W_d = nc.dram_tensor("W", [NB, LC, 3 * D], f32, kind="ExternalInput").ap()
    O_d = nc.dram_tensor("OUT", [NB, 4 * D, LC], f32, kind="ExternalOutput").ap()

    with tile.TileContext(nc) as tc, ExitStack() as ctx:
        const_pool = ctx.enter_context(tc.tile_pool(name="const", bufs=1))
        ident = const_pool.tile([128, 128], f32)
        masks.make_identity(nc, ident[:])
        identr = const_pool.tile([128, 128], f32r)
        nc.scalar.activation(identr[:], ident[:], AF.Copy)
        # f32r ones for the s1 column-sum / broadcast matmuls
        ones_f = const_pool.tile([128, 1], f32)
        nc.vector.memset(ones_f[:], 1.0)
        ones_col = const_pool.tile([128, 1], f32r)
        nc.scalar.activation(ones_col[:], ones_f[:], AF.Copy)
        ones_rf = const_pool.tile([1, 128], f32)
        nc.vector.memset(ones_rf[:], 1.0)
        ones_row = const_pool.tile([1, 128], f32r)
        nc.scalar.activation(ones_row[:], ones_rf[:], AF.Copy)
        zero_f = const_pool.tile([128, 1], f32)
        nc.vector.memset(zero_f[:], 0.0)

        sb = ctx.enter_context(tc.tile_pool(name="sb", bufs=2))
        small = ctx.enter_context(tc.tile_pool(name="small", bufs=3))
        outp = ctx.enter_context(tc.tile_pool(name="outp", bufs=2))
        tp_ps = ctx.enter_context(tc.tile_pool(name="tp_ps", bufs=3, space="PSUM"))
        mm_ps = ctx.enter_context(tc.tile_pool(name="mm_ps", bufs=5, space="PSUM"))

        for b in range(NB):
            # ---- loads ----
            # w_sb[p, t, c] = W[b, p*8+t, c]  (12KB contiguous per partition)
            w_sb = sb.tile([TL, NT * 3 * D], f32, tag="w", name=f"w{b}", bufs=3)
            nc.gpsimd.dma_start(
                w_sb[:].rearrange("p (t c) -> p t c", c=3 * D),
                W_d[b].rearrange("(p t) c -> p t c", t=NT))
            c_sb = sb.tile([D, LC], f32, tag="c", name=f"c{b}", bufs=3)
            nc.sync.dma_start(c_sb[:], C_d[b])
            q_sb = sb.tile([D, LQ], f32, tag="q", name=f"q{b}", bufs=3)
            nc.sync.dma_start(q_sb[:], Q_d[b])

            # ---- Qt (early: only needs the Q load) ----
            qtp = tp_ps.tile([LQ, D], f32, tag="tp", name=f"qtp{b}")
            nc.tensor.transpose(qtp[:], q_sb[:], ident[:])
            qt_sb = small.tile([LQ, D], f32r, tag="qt", name=f"qt{b}")
            nc.vector.tensor_copy(qt_sb[:], qtp[:])
            q_r = small.tile([D, LQ], f32r, tag="q_r", name=f"q_r{b}")
            nc.vector.tensor_copy(q_r[:], q_sb[:])

            # views with the interleaved Lc mapping  i = p*8 + t
            c_r = sb.tile([D, LC], f32r, tag="c_r", name=f"c_r{b}")
            nc.scalar.activation(c_r[:], c_sb[:], AF.Copy)
            c_tiles = c_r[:].rearrange("d (p t) -> d t p", t=NT)  # [D, t, p]
            wv = w_sb[:].rearrange("p (t c) -> p t c", c=3 * D)
            w1 = wv[:, :, 0:D]
            w2 = wv[:, :, D:2 * D]
            w3 = wv[:, :, 2 * D:3 * D]

            # ---- Ct tiles: PE transpose groups of 4; keep PSUM f32 copy
            #      for U/v, write bf16 (+ones col) SBUF copy for Tu ----
            ct_sb = sb.tile([TL, NT * (D + 1)], ct_dt, tag="ct", name=f"ct{b}")
            ctv = ct_sb[:].rearrange("p (t c) -> p t c", c=D + 1)
            nc.vector.memset(ctv[:, :, D:D + 1], 1.0)
            w3ct = sb.tile([TL, NT * D], f32, tag="w3ct", name=f"w3ct{b}")
            w3ctv = w3ct[:].rearrange("p (t c) -> p t c", c=D)
            vtmp = sb.tile([TL, NT * D], f32, tag="vtmp", name=f"vtmp{b}")
            vtmpv = vtmp[:].rearrange("p (t c) -> p t c", c=D)
            for g in range(2):
                ctp = tp_ps.tile([TL, 4 * D], f32r, tag="tp", name=f"ctp{b}_{g}")
                for k in range(4):
                    t = 4 * g + k
                    nc.tensor.transpose(
                        ctp[:, k * D:(k + 1) * D],
                        c_tiles[:, t, :], identr[:])
                ctpv = ctp[:].bitcast(f32).rearrange("p (k c) -> p k c", c=D)
                gs = slice(4 * g, 4 * g + 4)
                nc.scalar.activation(ctv[:, gs, 0:D], ctpv, AF.Copy)
                nc.vector.tensor_tensor(out=w3ctv[:, gs, :], in0=w3[:, gs, :],
                                        in1=ctpv, op=ALU.mult)
                if not v_bf16:
                    nc.vector.tensor_tensor(out=vtmpv[:, gs, :],
                                            in0=w2[:, gs, :], in1=ctpv,
                                            op=ALU.mult)
            if v_bf16:
                # gpsimd is idle; feed it the v multiply from the bf16 ct
                nc.gpsimd.tensor_tensor(out=vtmpv, in0=w2,
                                        in1=ctv[:, :, 0:D], op=ALU.mult)
            v_all = small.tile([TL, NT], f32, tag="v", name=f"v{b}")
            nc.vector.tensor_reduce(v_all[:], vtmpv, axis=AX.X, op=ALU.add)
            expv = small.tile([TL, NT], f32, tag="expv", name=f"expv{b}")
            nc.scalar.activation(expv[:], v_all[:], AF.Exp)

            # ---- U^T via PSUM-accumulating PE transposes:
            #      utp = w1_tile^T  (+)  (w3*Ct)_tile^T  ----
            ut_sb = sb.tile([D, 8 * D], f32r, tag="ut", name=f"ut{b}")
            utv = ut_sb[:].rearrange("d (t c) -> d t c", c=D)
            nc.scalar.activation(
                utv[:, :, TL:D],
                zero_f[:, 0:1].to_broadcast((D, NT, D - TL)), AF.Copy)
            e1t_sb = sb.tile([LQ, 8 * D], f32r, tag="e1t", name=f"e1t{b}")
            s0h = []
            u_all = sb.tile([D, NT * D], f32r, tag="u", name=f"u{b}")
            nc.scalar.activation(
                u_all[96:D, :].rearrange("p (t c) -> p t c", c=D),
                zero_f[96:D, 0:1].to_broadcast((D - 96, NT, D)), AF.Copy)
            nc.vector.tensor_tensor(
                out=u_all[0:TL, :].rearrange("p (t c) -> p t c", c=D),
                in0=w3ctv, in1=w1, op=ALU.add)
            for g in range(2):
                utp = tp_ps.tile([D, 4 * D], f32r, tag="tp", name=f"utp{b}_{g}")
                for k in range(4):
                    t = 4 * g + k
                    nc.tensor.transpose(utp[:, k * D:(k + 1) * D],
                                        u_all[:, t * D:(t + 1) * D],
                                        identr[:])
                nc.scalar.activation(
                    utv[:, 4 * g:4 * g + 4, 0:TL],
                    utp[:].rearrange("d (k c) -> d k c", c=D)[:, :, 0:TL],
                    AF.Copy)
                s0g = mm_ps.tile([LQ, 512], f32, tag="mmh", name=f"s0t{b}_{g}")
                s0h.append(s0g)
                nc.tensor.matmul(s0g[:], q_r[:],
                                 ut_sb[:, g * 512:(g + 1) * 512],
                                 start=True, stop=True)
                nc.scalar.activation(e1t_sb[:, g * 512:(g + 1) * 512],
                                     s0g[:], AF.Exp)

            # ---- E2 tiles = transpose(E1T) * exp(v); row-sums of E1 tiles
            #      feed the s1 normalization ----
            e2_all = sb.tile([TL, NT * LQ], e2_dt, tag="e2", name=f"e2{b}")
            e2v = e2_all[:].rearrange("p (t c) -> p t c", c=LQ)
            s1a = small.tile([TL, NT], f32, tag="s1a", name=f"s1a{b}")
            for g in range(2):
                e1p = tp_ps.tile([TL, 4 * LQ], f32r, tag="tp",
                                 name=f"e1p{b}_{g}")
                for k in range(4):
                    t = 4 * g + k
                    nc.tensor.transpose(
                        e1p[:, k * LQ:(k + 1) * LQ],
                        e1t_sb[:, t * D:t * D + TL],
                        identr[0:LQ, 0:LQ])
                e1pv = e1p[:].rearrange("p (k c) -> p k c", c=LQ)
                scl = expv[:, 4 * g:4 * g + 4].unsqueeze(-1).to_broadcast(
                    (TL, 4, LQ))
                nc.vector.tensor_tensor(
                    out=e2v[:, 4 * g:4 * g + 4, :], in0=e1pv,
                    in1=scl, op=ALU.mult)
                nc.vector.tensor_reduce(s1a[:, 4 * g:4 * g + 4], e1pv,
                                        axis=AX.X, op=ALU.add)

            # ---- 1/s1 -> [1, 1024] row -> partition broadcast ----
            s1ra = small.tile([D, NT], f32, tag="s1ra", name=f"s1ra{b}")
            nc.vector.memset(s1ra[96:D, :], 1.0)
            nc.vector.reciprocal(s1ra[0:TL, :], s1a[:])
            s1rp = tp_ps.tile([NT, D], f32, tag="tp", name=f"s1rp{b}")
            nc.tensor.transpose(s1rp[:], s1ra[:], ident[:])
            s1st = small.tile([NT, D], f32r, tag="s1st", name=f"s1st{b}")
            nc.vector.tensor_copy(s1st[:], s1rp[:])
            s1row = small.tile([1, 8 * D], f32r, tag="s1row", name=f"s1row{b}")
            nc.sync.dma_start(s1row[:], s1st[:])
            s1bb = sb.tile([D, 8 * D], f32, tag="s1bb", name=f"s1bb{b}")
            nc.gpsimd.partition_broadcast(s1bb[:], s1row[:].bitcast(f32),
                                          channels=D)

            # ---- Tu = E2^T @ [Ct | 1]  (accumulate over tiles) ----
            tu = tp_ps.tile([LQ, D + 1], f32, tag="tp", name=f"tu{b}")
            for t in range(NT):
                nc.tensor.matmul(tu[:], e2v[:, t, :], ctv[:, t, :],
                                 start=(t == 0), stop=(t == NT - 1))
            s2r = small.tile([LQ, 1], f32, tag="s2r", name=f"s2r{b}")
            nc.vector.reciprocal(s2r[:], tu[:, D:D + 1])
            that_sb = small.tile([LQ, D], f32r, tag="that", name=f"that{b}")
            nc.vector.tensor_scalar_mul(that_sb[:], tu[:, 0:D], s2r[:])

            # ---- A^T and Bm^T (per half) + unpermuted outputs ----
            cpt = c_sb[:].rearrange("d (p t) -> d p t", t=NT)
            cs1 = outp.tile([D, LC], f32, tag="cs1", name=f"cs1{b}")
            # cs1[d, (p t)] = C[d,i] / s1[i]  (s1bb holds 1/s1 in the
            # permuted 128-pitch column space)
            nc.vector.tensor_tensor(
                out=cs1[:].rearrange("d (p t) -> d p t", t=NT),
                in0=cpt,
                in1=s1bb[:].rearrange("d (t c) -> d c t", c=D)[:, 0:TL, :],
                op=ALU.mult)
            cs1v = cs1[:].rearrange("d (p t) -> d p t", t=NT)
            oa = outp.tile([D, LC], f32, tag="oa", name=f"oa{b}")
            oca = outp.tile([D, LC], f32, tag="oca", name=f"oca{b}")
            ocb = outp.tile([D, LC], f32, tag="ocb", name=f"ocb{b}")
            nc.sync.dma_start(O_d[b, 0:D], c_sb[:])
            for g in range(2):
                gsl = slice(g * 512, (g + 1) * 512)
                tsl = slice(4 * g, 4 * g + 4)
                ath = mm_ps.tile([D, 512], f32, tag="mmh", name=f"at{b}_{g}")
                nc.tensor.matmul(ath[:], qt_sb[:], e1t_sb[:, gsl],
                                 start=True, stop=True)
                bmh = mm_ps.tile([D, 512], f32, tag="mmh", name=f"bm{b}_{g}")
                nc.tensor.matmul(bmh[:], that_sb[:], e1t_sb[:, gsl],
                                 start=True, stop=True)
                athp = ath[:].rearrange("d (t c) -> d c t", c=D)[:, 0:TL, :]
                bmhp = bmh[:].rearrange("d (t c) -> d c t", c=D)[:, 0:TL, :]
                s1bp = s1bb[:, gsl].rearrange(
                    "d (t c) -> d c t", c=D)[:, 0:TL, :]
                oav = oa[:].rearrange("d (p t) -> d p t", t=NT)[:, :, tsl]
                ocav = oca[:].rearrange("d (p t) -> d p t", t=NT)[:, :, tsl]
                ocbv = ocb[:].rearrange("d (p t) -> d p t", t=NT)[:, :, tsl]
                nc.vector.tensor_tensor(out=oav, in0=athp, in1=s1bp,
                                        op=ALU.mult)
                nc.vector.tensor_tensor(out=ocav, in0=athp,
                                        in1=cs1v[:, :, tsl], op=ALU.mult)
                nc.vector.tensor_tensor(out=ocbv, in0=bmhp,
                                        in1=cs1v[:, :, tsl], op=ALU.mult)
            nc.sync.dma_start(O_d[b, D:2 * D], oa[:])
            nc.sync.dma_start(O_d[b, 2 * D:3 * D], oca[:])
            nc.sync.dma_start(O_d[b, 3 * D:4 * D], ocb[:])

    nc.compile()
    return nc


def _get_nc(**kw):
    key = tuple(sorted(kw.items()))
    if key not in _cache:
        _cache[key] = _build(**kw)
    return _cache[key]


def kernel(C, Q, W, **build_kw):
    from concourse import bass_utils

    C = np.ascontiguousarray(C, np.float32)
    Q = np.ascontiguousarray(Q, np.float32)
    Wr = np.ascontiguousarray(W, np.float32).reshape(NCORES, NB, LC, 3 * D)
    Cs = C.reshape(NCORES, NB, D, LC)
    Qs = Q.reshape(NCORES, NB, D, LQ)

    nc = _get_nc(**build_kw)
    in_maps = [{"C": Cs[i], "Q": Qs[i], "W": Wr[i]} for i in range(NCORES)]
    res = bass_utils.run_bass_kernel_spmd(nc, in_maps,
                                          core_ids=list(range(NCORES)))
    out = np.concatenate([res.results[i]["OUT"] for i in range(NCORES)], 0)
    return out.astype(np.float32)


# revision 27
# speedup vs baseline: 1.2517x; 1.0241x over previous
"""CQAttention (context-query attention) Bass kernel for 8 NeuronCores.

Full inputs:  C [64,128,1000] f32, Q [64,128,100] f32, W [64000,1,384] f32
Full output:  [64, 512, 1000] f32

Sharding: pure data-parallel on the batch dim - 8 batches per core.

Per-batch math (D=128, Lc=1000, Lq=100):
  Ct = C.T [Lc,D], Qt = Q.T [Lq,D], w1/w2/w3 = W row blocks [Lc,D]
  U  = w1 + w3*Ct ; v = rowsum(w2*Ct)
  S  = U @ Q + v  (the v term drops out of the row softmax S1)
  S1 = softmax_cols(S) ; S2 = softmax_rows(S)
  A  = S1 @ Qt ; Bm = S1 @ (S2^T @ Ct)
  out = concat([Ct, A, Ct*A, Ct*Bm], 1).T  -> [4D, Lc]

Layout notes:
 - Lc is tiled 8 x 125 with the INTERLEAVED mapping i = p*8 + t (p =
   partition, t = tile) so the W DMA reads 12KB contiguous per partition.
   All intermediate tensors with an Lc axis are kept in the permuted
   (t-major) order; the final output ops unpermute via strided APs.
 - Scores are built transposed (S0T [Lq, Lc]) so the big matmuls run with
   float32r operands at full PE rate (moving free dim >= 256).
 - S1 normalization: column sums of exp(S0T) via a ones-vector matmul,
   reciprocal, then a K=1 matmul broadcast across partitions.
 - S2 path: PE-transpose exp(S0T) tiles, scale by exp(v), cast bf16, and
   contract with bf16 Ct tiles (ones column appended for the s2 sums).
"""

import numpy as np

B, D, LC, LQ = 64, 128, 1000, 100
NCORES = 8
NB = B // NCORES   # batches per core
NT = 8             # LC tiles
TL = LC // NT      # 125

_cache = {}


def _build(tu_bf16=True, v_bf16=True):
    import concourse.bass as bass
    import concourse.tile as tile
    from concourse import bacc, mybir, masks
    from contextlib import ExitStack

    f32 = mybir.dt.float32
    f32r = mybir.dt.float32r
    bf16 = mybir.dt.bfloat16
    AF = mybir.ActivationFunctionType
    ALU = mybir.AluOpType
    AX = mybir.AxisListType

    ct_dt = bf16 if tu_bf16 else f32
    e2_dt = bf16 if tu_bf16 else f32

    nc = bacc.Bacc("TRN2", target_bir_lowering=False, debug=False,
                   num_devices=NCORES)
    C_d = nc.dram_tensor("C", [NB, D, LC], f32, kind="ExternalInput").ap()
    Q_d = nc.dram_tensor("Q", [NB, D, LQ], f32, kind="ExternalInput").ap()
    W_d = nc.dram_tensor("W", [NB, LC, 3 * D], f32, kind="ExternalInput").ap()
    O_d = nc.dram_tensor("OUT", [NB, 4 * D, LC], f32, kind="ExternalOutput").ap()

    with tile.TileContext(nc) as tc, ExitStack() as ctx:
        const_pool = ctx.enter_context(tc.tile_pool(name="const", bufs=1))
        ident = const_pool.tile([128, 128], f32)
        masks.make_identity(nc, ident[:])
        identr = const_pool.tile([128, 128], f32r)
        nc.scalar.activation(identr[:], ident[:], AF.Copy)
        # f32r ones for the s1 column-sum / broadcast matmuls
        ones_f = const_pool.tile([128, 1], f32)
        nc.vector.memset(ones_f[:], 1.0)
        ones_col = const_pool.tile([128, 1], f32r)
        nc.scalar.activation(ones_col[:], ones_f[:], AF.Copy)
        ones_rf = const_pool.tile([1, 128], f32)
        nc.vector.memset(ones_rf[:], 1.0)
        ones_row = const_pool.tile([1, 128], f32r)
        nc.scalar.activation(ones_row[:], ones_rf[:], AF.Copy)
        zero_f = const_pool.tile([128, 1], f32)
        nc.vector.memset(zero_f[:], 0.0)

        sb = ctx.enter_context(tc.tile_pool(name="sb", bufs=2))
        small = ctx.enter_context(tc.tile_pool(name="small", bufs=3))
        outp = ctx.enter_context(tc.tile_pool(name="outp", bufs=2))
        tp_ps = ctx.enter_context(tc.tile_pool(name="tp_ps", bufs=3, space="PSUM"))
        mm_ps = ctx.enter_context(tc.tile_pool(name="mm_ps", bufs=5, space="PSUM"))

        for b in range(NB):
            # ---- loads ----
            # w_sb[p, t, c] = W[b, p*8+t, c]  (12KB contiguous per partition)
            w_sb = sb.tile([TL, NT * 3 * D], f32, tag="w", name=f"w{b}", bufs=3)
            nc.gpsimd.dma_start(
                w_sb[:].rearrange("p (t c) -> p t c", c=3 * D),
                W_d[b].rearrange("(p t) c -> p t c", t=NT))
            c_sb = sb.tile([D, LC], f32, tag="c", name=f"c{b}", bufs=3)
            nc.sync.dma_start(c_sb[:], C_d[b])
            q_sb = sb.tile([D, LQ], f32, tag="q", name=f"q{b}", bufs=3)
            nc.sync.dma_start(q_sb[:], Q_d[b])

            # ---- Qt (early: only needs the Q load) ----
            qtp = tp_ps.tile([LQ, D], f32, tag="tp", name=f"qtp{b}")
            nc.tensor.transpose(qtp[:], q_sb[:], ident[:])
            qt_sb = small.tile([LQ, D], f32r, tag="qt", name=f"qt{b}")
            nc.vector.tensor_copy(qt_sb[:], qtp[:])
            q_r = small.tile([D, LQ], f32r, tag="q_r", name=f"q_r{b}")
            nc.vector.tensor_copy(q_r[:], q_sb[:])

            # views with the interleaved Lc mapping  i = p*8 + t
            c_r = sb.tile([D, LC], f32r, tag="c_r", name=f"c_r{b}")
            nc.scalar.activation(c_r[:], c_sb[:], AF.Copy)
            c_tiles = c_r[:].rearrange("d (p t) -> d t p", t=NT)  # [D, t, p]
            wv = w_sb[:].rearrange("p (t c) -> p t c", c=3 * D)
            w1 = wv[:, :, 0:D]
            w2 = wv[:, :, D:2 * D]
            w3 = wv[:, :, 2 * D:3 * D]

            # ---- Ct tiles: PE transpose groups of 4; keep PSUM f32 copy
            #      for U/v, write bf16 (+ones col) SBUF copy for Tu ----
            ct_sb = sb.tile([TL, NT * (D + 1)], ct_dt, tag="ct", name=f"ct{b}")
            ctv = ct_sb[:].rearrange("p (t c) -> p t c", c=D + 1)
            nc.vector.memset(ctv[:, :, D:D + 1], 1.0)
            w3ct = sb.tile([TL, NT * D], f32, tag="w3ct", name=f"w3ct{b}")
            w3ctv = w3ct[:].rearrange("p (t c) -> p t c", c=D)
            vtmp = sb.tile([TL, NT * D], f32, tag="vtmp", name=f"vtmp{b}")
            vtmpv = vtmp[:].rearrange("p (t c) -> p t c", c=D)
            for g in range(2):
                ctp = tp_ps.tile([TL, 4 * D], f32r, tag="tp", name=f"ctp{b}_{g}")
                for k in range(4):
                    t = 4 * g + k
                    nc.tensor.transpose(
                        ctp[:, k * D:(k + 1) * D],
                        c_tiles[:, t, :], identr[:])
                ctpv = ctp[:].bitcast(f32).rearrange("p (k c) -> p k c", c=D)
                gs = slice(4 * g, 4 * g + 4)
                nc.scalar.activation(ctv[:, gs, 0:D], ctpv, AF.Copy)
                nc.vector.tensor_tensor(out=w3ctv[:, gs, :], in0=w3[:, gs, :],
                                        in1=ctpv, op=ALU.mult)
                if not v_bf16:
                    nc.vector.tensor_tensor(out=vtmpv[:, gs, :],
                                            in0=w2[:, gs, :], in1=ctpv,
                                            op=ALU.mult)
            if v_bf16:
                # gpsimd is idle; feed it the v multiply from the bf16 ct
                nc.gpsimd.tensor_tensor(out=vtmpv, in0=w2,
                                        in1=ctv[:, :, 0:D], op=ALU.mult)
            v_all = small.tile([TL, NT], f32, tag="v", name=f"v{b}")
            nc.vector.tensor_reduce(v_all[:], vtmpv, axis=AX.X, op=ALU.add)
            expv = small.tile([TL, NT], f32, tag="expv", name=f"expv{b}")
            nc.scalar.activation(expv[:], v_all[:], AF.Exp)
            env = small.tile([TL, NT], f32, tag="env", name=f"env{b}")
            nc.scalar.activation(env[:], v_all[:], AF.Exp, scale=-1.0)

            # ---- U^T via PSUM-accumulating PE transposes:
            #      utp = w1_tile^T  (+)  (w3*Ct)_tile^T  ----
            ut_sb = sb.tile([D, 8 * D], f32r, tag="ut", name=f"ut{b}")
            utv = ut_sb[:].rearrange("d (t c) -> d t c", c=D)
            nc.scalar.activation(
                utv[:, :, TL:D],
                zero_f[:, 0:1].to_broadcast((D, NT, D - TL)), AF.Copy)
            e1t_sb = sb.tile([LQ, 8 * D], f32r, tag="e1t", name=f"e1t{b}")
            s0h = []
            for g in range(2):
                utp = tp_ps.tile([D, 4 * TL], f32, tag="tp", name=f"utp{b}_{g}")
                for k in range(4):
                    t = 4 * g + k
                    nc.tensor.matmul(utp[:, k * TL:(k + 1) * TL],
                                     w_sb[:, t * 3 * D:t * 3 * D + D],
                                     ident[0:TL, 0:TL],
                                     is_transpose=True, start=True, stop=False)
                    nc.tensor.matmul(utp[:, k * TL:(k + 1) * TL],
                                     w3ct[:, t * D:(t + 1) * D],
                                     ident[0:TL, 0:TL],
                                     is_transpose=True, start=False, stop=True)
                nc.scalar.activation(
                    utv[:, 4 * g:4 * g + 4, 0:TL],
                    utp[:].rearrange("d (k c) -> d k c", c=TL), AF.Copy)
                s0g = mm_ps.tile([LQ, 512], f32, tag="mmh", name=f"s0t{b}_{g}")
                s0h.append(s0g)
                nc.tensor.matmul(s0g[:], q_r[:],
                                 ut_sb[:, g * 512:(g + 1) * 512],
                                 start=True, stop=True)
                nc.scalar.activation(e1t_sb[:, g * 512:(g + 1) * 512],
                                     s0g[:], AF.Exp)

            # ---- E2 tiles = transpose(E1T) * exp(v); row-sums of E1 tiles
            #      feed the s1 normalization ----
            e2_all = sb.tile([TL, NT * LQ], e2_dt, tag="e2", name=f"e2{b}")
            e2v = e2_all[:].rearrange("p (t c) -> p t c", c=LQ)
            s1a = small.tile([TL, NT], f32, tag="s1a", name=f"s1a{b}")
            e2rs = small.tile([TL, NT], f32, tag="e2rs", name=f"e2rs{b}")
            for g in range(2):
                e1p = tp_ps.tile([TL, 4 * LQ], f32r, tag="tp",
                                 name=f"e1p{b}_{g}")
                for k in range(4):
                    t = 4 * g + k
                    nc.tensor.transpose(
                        e1p[:, k * LQ:(k + 1) * LQ],
                        e1t_sb[:, t * D:t * D + TL],
                        identr[0:LQ, 0:LQ])
                for k in range(4):
                    t = 4 * g + k
                    nc.scalar.activation(
                        e2v[:, t, :], e1p[:, k * LQ:(k + 1) * LQ], AF.Copy,
                        scale=expv[:, t:t + 1], accum_out=e2rs[:, t:t + 1])

            # ---- 1/s1 -> [1, 1024] row -> partition broadcast ----
            nc.vector.tensor_tensor(out=s1a[:], in0=e2rs[:], in1=env[:],
                                     op=ALU.mult)
            s1ra = small.tile([D, NT], f32, tag="s1ra", name=f"s1ra{b}")
            nc.vector.memset(s1ra[96:D, :], 1.0)
            nc.vector.reciprocal(s1ra[0:TL, :], s1a[:])
            s1rp = tp_ps.tile([NT, D], f32, tag="tp", name=f"s1rp{b}")
            nc.tensor.transpose(s1rp[:], s1ra[:], ident[:])
            s1st = small.tile([NT, D], f32r, tag="s1st", name=f"s1st{b}")
            nc.vector.tensor_copy(s1st[:], s1rp[:])
            s1row = small.tile([1, 8 * D], f32r, tag="s1row", name=f"s1row{b}")
            nc.sync.dma_start(s1row[:], s1st[:])
            s1bb = sb.tile([D, 8 * D], f32, tag="s1bb", name=f"s1bb{b}")
            nc.gpsimd.partition_broadcast(s1bb[:], s1row[:].bitcast(f32),
                                          channels=D)

            # ---- Tu = E2^T @ [Ct | 1]  (accumulate over tiles) ----
            tu = tp_ps.tile([LQ, D + 1], f32, tag="tp", name=f"tu{b}")
            for t in range(NT):
                nc.tensor.matmul(tu[:], e2v[:, t, :], ctv[:, t, :],
                                 start=(t == 0), stop=(t == NT - 1))
            s2r = small.tile([LQ, 1], f32, tag="s2r", name=f"s2r{b}")
            nc.vector.reciprocal(s2r[:], tu[:, D:D + 1])
            that_sb = small.tile([LQ, D], f32r, tag="that", name=f"that{b}")
            nc.vector.tensor_scalar_mul(that_sb[:], tu[:, 0:D], s2r[:])

            # ---- A^T and Bm^T (per half) + unpermuted outputs ----
            cpt = c_sb[:].rearrange("d (p t) -> d p t", t=NT)
            cs1 = outp.tile([D, LC], f32, tag="cs1", name=f"cs1{b}")
            # cs1[d, (p t)] = C[d,i] / s1[i]  (s1bb holds 1/s1 in the
            # permuted 128-pitch column space)
            nc.vector.tensor_tensor(
                out=cs1[:].rearrange("d (p t) -> d p t", t=NT),
                in0=cpt,
                in1=s1bb[:].rearrange("d (t c) -> d c t", c=D)[:, 0:TL, :],
                op=ALU.mult)
            cs1v = cs1[:].rearrange("d (p t) -> d p t", t=NT)
            oa = outp.tile([D, LC], f32, tag="oa", name=f"oa{b}")
            oca = outp.tile([D, LC], f32, tag="oca", name=f"oca{b}")
            ocb = outp.tile([D, LC], f32, tag="ocb", name=f"ocb{b}")
            nc.sync.dma_start(O_d[b, 0:D], c_sb[:])
            for g in range(2):
                gsl = slice(g * 512, (g + 1) * 512)
                tsl = slice(4 * g, 4 * g + 4)
                ath = mm_ps.tile([D, 512], f32, tag="mmh", name=f"at{b}_{g}")
                nc.tensor.matmul(ath[:], qt_sb[:], e1t_sb[:, gsl],
                                 start=True, stop=True)
                bmh = mm_ps.tile([D, 512], f32, tag="mmh", name=f"bm{b}_{g}")
                nc.tensor.matmul(bmh[:], that_sb[:], e1t_sb[:, gsl],
                                 start=True, stop=True)
                athp = ath[:].rearrange("d (t c) -> d c t", c=D)[:, 0:TL, :]
                bmhp = bmh[:].rearrange("d (t c) -> d c t", c=D)[:, 0:TL, :]
                s1bp = s1bb[:, gsl].rearrange(
                    "d (t c) -> d c t", c=D)[:, 0:TL, :]
                oav = oa[:].rearrange("d (p t) -> d p t", t=NT)[:, :, tsl]
                ocav = oca[:].rearrange("d (p t) -> d p t", t=NT)[:, :, tsl]
                ocbv = ocb[:].rearrange("d (p t) -> d p t", t=NT)[:, :, tsl]
                nc.vector.tensor_tensor(out=oav, in0=athp, in1=s1bp,
                                        op=ALU.mult)
                nc.vector.tensor_tensor(out=ocav, in0=athp,
                                        in1=cs1v[:, :, tsl], op=ALU.mult)
                nc.vector.tensor_tensor(out=ocbv, in0=bmhp,
                                        in1=cs1v[:, :, tsl], op=ALU.mult)
            nc.sync.dma_start(O_d[b, D:2 * D], oa[:])
            nc.sync.dma_start(O_d[b, 2 * D:3 * D], oca[:])
            nc.sync.dma_start(O_d[b, 3 * D:4 * D], ocb[:])

    nc.compile()
    return nc


def _get_nc(**kw):
    key = tuple(sorted(kw.items()))
    if key not in _cache:
        _cache[key] = _build(**kw)
    return _cache[key]


def kernel(C, Q, W, **build_kw):
    from concourse import bass_utils

    C = np.ascontiguousarray(C, np.float32)
    Q = np.ascontiguousarray(Q, np.float32)
    Wr = np.ascontiguousarray(W, np.float32).reshape(NCORES, NB, LC, 3 * D)
    Cs = C.reshape(NCORES, NB, D, LC)
    Qs = Q.reshape(NCORES, NB, D, LQ)

    nc = _get_nc(**build_kw)
    in_maps = [{"C": Cs[i], "Q": Qs[i], "W": Wr[i]} for i in range(NCORES)]
    res = bass_utils.run_bass_kernel_spmd(nc, in_maps,
                                          core_ids=list(range(NCORES)))
    out = np.concatenate([res.results[i]["OUT"] for i in range(NCORES)], 0)
    return out.astype(np.float32)


# revision 28
# speedup vs baseline: 1.3121x; 1.0483x over previous
"""CQAttention (context-query attention) Bass kernel for 8 NeuronCores.

Full inputs:  C [64,128,1000] f32, Q [64,128,100] f32, W [64000,1,384] f32
Full output:  [64, 512, 1000] f32

Sharding: pure data-parallel on the batch dim - 8 batches per core.

Per-batch math (D=128, Lc=1000, Lq=100):
  Ct = C.T [Lc,D], Qt = Q.T [Lq,D], w1/w2/w3 = W row blocks [Lc,D]
  U  = w1 + w3*Ct ; v = rowsum(w2*Ct)
  S  = U @ Q + v  (the v term drops out of the row softmax S1)
  S1 = softmax_cols(S) ; S2 = softmax_rows(S)
  A  = S1 @ Qt ; Bm = S1 @ (S2^T @ Ct)
  out = concat([Ct, A, Ct*A, Ct*Bm], 1).T  -> [4D, Lc]

Implementation notes:
 - Lc is tiled 8 x 125 with the INTERLEAVED mapping i = p*8 + t so the W
   DMA reads 12KB contiguous per partition; intermediates stay in the
   permuted (t-major, 128-pitch) column space and the output ops
   unpermute via strided APs.
 - Scores are built transposed (S0T [Lq, 1024]) with float32r operands.
 - U^T is built by PSUM-accumulating PE transposes (w1^T + (w3*Ct)^T).
 - S1 normalization: per-tile row sums (ACT accum) -> 1/s1 [125,8] ->
   PE transpose -> SBUF row -> gpsimd partition-broadcast; it is applied
   in the output stage so it never gates the A/Bm matmuls.
 - A^T/Bm^T are computed unnormalized against exp(S0T).
 - Emission is software-pipelined: stage A(b) | B(b-1) | C(b-2) so each
   engine's in-order instruction stream interleaves batches.
"""

import numpy as np

B, D, LC, LQ = 64, 128, 1000, 100
NCORES = 8
NB = B // NCORES   # batches per core
NT = 8             # LC tiles
TL = LC // NT      # 125

_cache = {}


def _build(tu_bf16=True):
    import concourse.bass as bass
    import concourse.tile as tile
    from concourse import bacc, mybir, masks
    from contextlib import ExitStack

    f32 = mybir.dt.float32
    f32r = mybir.dt.float32r
    bf16 = mybir.dt.bfloat16
    AF = mybir.ActivationFunctionType
    ALU = mybir.AluOpType
    AX = mybir.AxisListType

    ct_dt = bf16 if tu_bf16 else f32
    e2_dt = bf16 if tu_bf16 else f32

    nc = bacc.Bacc("TRN2", target_bir_lowering=False, debug=False,
                   num_devices=NCORES)
    C_d = nc.dram_tensor("C", [NB, D, LC], f32, kind="ExternalInput").ap()
    Q_d = nc.dram_tensor("Q", [NB, D, LQ], f32, kind="ExternalInput").ap()
    W_d = nc.dram_tensor("W", [NB, LC, 3 * D], f32, kind="ExternalInput").ap()
    O_d = nc.dram_tensor("OUT", [NB, 4 * D, LC], f32, kind="ExternalOutput").ap()

    with tile.TileContext(nc) as tc, ExitStack() as ctx:
        const_pool = ctx.enter_context(tc.tile_pool(name="const", bufs=1))
        ident = const_pool.tile([128, 128], f32)
        masks.make_identity(nc, ident[:])
        identr = const_pool.tile([128, 128], f32r)
        nc.scalar.activation(identr[:], ident[:], AF.Copy)
        zero_f = const_pool.tile([128, 1], f32)
        nc.vector.memset(zero_f[:], 0.0)

        sb = ctx.enter_context(tc.tile_pool(name="sb", bufs=2))
        small = ctx.enter_context(tc.tile_pool(name="small", bufs=3))
        outp = ctx.enter_context(tc.tile_pool(name="outp", bufs=2))
        pa_ps = ctx.enter_context(tc.tile_pool(name="pa_ps", bufs=3, space="PSUM"))
        pb_ps = ctx.enter_context(tc.tile_pool(name="pb_ps", bufs=2, space="PSUM"))
        pc_ps = ctx.enter_context(tc.tile_pool(name="pc_ps", bufs=3, space="PSUM"))

        st = [dict() for _ in range(NB)]

        def stage_a(b):
            s = st[b]
            w_sb = sb.tile([TL, NT * 3 * D], f32, tag="w", name=f"w{b}")
            nc.gpsimd.dma_start(
                w_sb[:].rearrange("p (t c) -> p t c", c=3 * D),
                W_d[b].rearrange("(p t) c -> p t c", t=NT))
            c_sb = sb.tile([D, LC], f32, tag="c", name=f"c{b}", bufs=4)
            nc.sync.dma_start(c_sb[:], C_d[b])
            q_sb = sb.tile([D, LQ], f32, tag="q", name=f"q{b}")
            nc.sync.dma_start(q_sb[:], Q_d[b])
            s["c_sb"] = c_sb

            qtp = pa_ps.tile([LQ, D], f32, tag="pa", name=f"qtp{b}")
            nc.tensor.transpose(qtp[:], q_sb[:], ident[:])
            qt_sb = small.tile([LQ, D], f32r, tag="qt", name=f"qt{b}", bufs=4)
            nc.vector.tensor_copy(qt_sb[:], qtp[:])
            q_r = small.tile([D, LQ], f32r, tag="q_r", name=f"q_r{b}")
            nc.vector.tensor_copy(q_r[:], q_sb[:])
            s["qt_sb"] = qt_sb

            c_r = sb.tile([D, LC], f32r, tag="c_r", name=f"c_r{b}")
            nc.scalar.activation(c_r[:], c_sb[:], AF.Copy)
            c_tiles = c_r[:].rearrange("d (p t) -> d t p", t=NT)
            wv = w_sb[:].rearrange("p (t c) -> p t c", c=3 * D)
            w2 = wv[:, :, D:2 * D]
            w3 = wv[:, :, 2 * D:3 * D]

            ct_sb = sb.tile([TL, NT * (D + 1)], ct_dt, tag="ct",
                            name=f"ct{b}", bufs=3)
            ctv = ct_sb[:].rearrange("p (t c) -> p t c", c=D + 1)
            nc.vector.memset(ctv[:, :, D:D + 1], 1.0)
            s["ctv"] = ctv
            w3ct = sb.tile([TL, NT * D], f32, tag="w3ct", name=f"w3ct{b}")
            w3ctv = w3ct[:].rearrange("p (t c) -> p t c", c=D)
            vtmp = sb.tile([TL, NT * D], f32, tag="vtmp", name=f"vtmp{b}")
            vtmpv = vtmp[:].rearrange("p (t c) -> p t c", c=D)
            for g in range(2):
                ctp = pa_ps.tile([TL, 4 * D], f32r, tag="pa",
                                 name=f"ctp{b}_{g}")
                for k in range(4):
                    t = 4 * g + k
                    nc.tensor.transpose(
                        ctp[:, k * D:(k + 1) * D], c_tiles[:, t, :], identr[:])
                ctpv = ctp[:].bitcast(f32).rearrange("p (k c) -> p k c", c=D)
                gs = slice(4 * g, 4 * g + 4)
                nc.scalar.activation(ctv[:, gs, 0:D], ctpv, AF.Copy)
                nc.vector.tensor_tensor(out=w3ctv[:, gs, :], in0=w3[:, gs, :],
                                        in1=ctpv, op=ALU.mult)
            # v = rowsum(w2 * Ct) from the bf16 Ct copy (gpsimd is idle)
            nc.gpsimd.tensor_tensor(out=vtmpv, in0=w2, in1=ctv[:, :, 0:D],
                                    op=ALU.mult)
            v_all = small.tile([TL, NT], f32, tag="v", name=f"v{b}")
            nc.vector.tensor_reduce(v_all[:], vtmpv, axis=AX.X, op=ALU.add)
            expv = small.tile([TL, NT], f32, tag="expv", name=f"expv{b}")
            nc.scalar.activation(expv[:], v_all[:], AF.Exp)
            env = small.tile([TL, NT], f32, tag="env", name=f"env{b}")
            nc.scalar.activation(env[:], v_all[:], AF.Exp, scale=-1.0)
            s["expv"], s["env"] = expv, env

            # U^T via PSUM-accumulating transposes: w1^T + (w3*Ct)^T
            ut_sb = sb.tile([D, 8 * D], f32r, tag="ut", name=f"ut{b}")
            utv = ut_sb[:].rearrange("d (t c) -> d t c", c=D)
            nc.scalar.activation(
                utv[:, :, TL:D],
                zero_f[:, 0:1].to_broadcast((D, NT, D - TL)), AF.Copy)
            e1t_sb = sb.tile([LQ, 8 * D], f32r, tag="e1t", name=f"e1t{b}",
                             bufs=4)
            for g in range(2):
                utp = pa_ps.tile([D, 4 * TL], f32, tag="pa", name=f"utp{b}_{g}")
                for k in range(4):
                    t = 4 * g + k
                    nc.tensor.matmul(utp[:, k * TL:(k + 1) * TL],
                                     w_sb[:, t * 3 * D:t * 3 * D + D],
                                     ident[0:TL, 0:TL],
                                     is_transpose=True, start=True, stop=False)
                    nc.tensor.matmul(utp[:, k * TL:(k + 1) * TL],
                                     w3ct[:, t * D:(t + 1) * D],
                                     ident[0:TL, 0:TL],
                                     is_transpose=True, start=False, stop=True)
                nc.scalar.activation(
                    utv[:, 4 * g:4 * g + 4, 0:TL],
                    utp[:].rearrange("d (k c) -> d k c", c=TL), AF.Copy)
                s0g = pa_ps.tile([LQ, 512], f32, tag="pa", name=f"s0t{b}_{g}")
                nc.tensor.matmul(s0g[:], q_r[:],
                                 ut_sb[:, g * 512:(g + 1) * 512],
                                 start=True, stop=True)
                nc.scalar.activation(e1t_sb[:, g * 512:(g + 1) * 512],
                                     s0g[:], AF.Exp)
            s["e1t_sb"] = e1t_sb

        def stage_b(b):
            s = st[b]
            e1t_sb, ctv = s["e1t_sb"], s["ctv"]
            expv, env = s["expv"], s["env"]
            e2_all = sb.tile([TL, NT * LQ], e2_dt, tag="e2", name=f"e2{b}",
                             bufs=3)
            e2v = e2_all[:].rearrange("p (t c) -> p t c", c=LQ)
            e2rs = small.tile([TL, NT], f32, tag="e2rs", name=f"e2rs{b}")
            for g in range(2):
                e1p = pb_ps.tile([TL, 4 * LQ], f32r, tag="pb",
                                 name=f"e1p{b}_{g}")
                for k in range(4):
                    t = 4 * g + k
                    nc.tensor.transpose(
                        e1p[:, k * LQ:(k + 1) * LQ],
                        e1t_sb[:, t * D:t * D + TL],
                        identr[0:LQ, 0:LQ])
                for k in range(4):
                    t = 4 * g + k
                    nc.scalar.activation(
                        e2v[:, t, :], e1p[:, k * LQ:(k + 1) * LQ], AF.Copy,
                        scale=expv[:, t:t + 1], accum_out=e2rs[:, t:t + 1])

            # 1/s1 row (permuted order) and its partition broadcast
            s1a = small.tile([TL, NT], f32, tag="s1a", name=f"s1a{b}")
            nc.vector.tensor_tensor(out=s1a[:], in0=e2rs[:], in1=env[:],
                                    op=ALU.mult)
            s1ra = small.tile([D, NT], f32, tag="s1ra", name=f"s1ra{b}")
            nc.vector.memset(s1ra[96:D, :], 1.0)
            nc.vector.reciprocal(s1ra[0:TL, :], s1a[:])
            s1rp = pb_ps.tile([NT, D], f32, tag="pb", name=f"s1rp{b}")
            nc.tensor.transpose(s1rp[:], s1ra[:], ident[:])
            s1st = small.tile([NT, D], f32, tag="s1st", name=f"s1st{b}")
            nc.vector.tensor_copy(s1st[:], s1rp[:])
            s1row = small.tile([1, 8 * D], f32, tag="s1row", name=f"s1row{b}")
            nc.sync.dma_start(s1row[:], s1st[:])
            s1bb = sb.tile([D, 8 * D], f32, tag="s1bb", name=f"s1bb{b}",
                           bufs=3)
            nc.gpsimd.partition_broadcast(s1bb[:], s1row[:], channels=D)
            s["s1bb"] = s1bb

            # Tu = E2^T @ [Ct | 1]
            tu = pb_ps.tile([LQ, D + 1], f32, tag="pb", name=f"tu{b}")
            for t in range(NT):
                nc.tensor.matmul(tu[:], e2v[:, t, :], ctv[:, t, :],
                                 start=(t == 0), stop=(t == NT - 1))
            s2r = small.tile([LQ, 1], f32, tag="s2r", name=f"s2r{b}")
            nc.vector.reciprocal(s2r[:], tu[:, D:D + 1])
            that_sb = small.tile([LQ, D], f32r, tag="that", name=f"that{b}")
            nc.vector.tensor_scalar_mul(that_sb[:], tu[:, 0:D], s2r[:])
            s["that_sb"] = that_sb

        def stage_c(b):
            s = st[b]
            c_sb, qt_sb, that_sb = s["c_sb"], s["qt_sb"], s["that_sb"]
            e1t_sb, s1bb = s["e1t_sb"], s["s1bb"]
            cpt = c_sb[:].rearrange("d (p t) -> d p t", t=NT)
            cs1 = outp.tile([D, LC], f32, tag="cs1", name=f"cs1{b}")
            nc.vector.tensor_tensor(
                out=cs1[:].rearrange("d (p t) -> d p t", t=NT),
                in0=cpt,
                in1=s1bb[:].rearrange("d (t c) -> d c t", c=D)[:, 0:TL, :],
                op=ALU.mult)
            cs1v = cs1[:].rearrange("d (p t) -> d p t", t=NT)
            oa = outp.tile([D, LC], f32, tag="oa", name=f"oa{b}")
            oca = outp.tile([D, LC], f32, tag="oca", name=f"oca{b}")
            ocb = outp.tile([D, LC], f32, tag="ocb", name=f"ocb{b}")
            nc.sync.dma_start(O_d[b, 0:D], c_sb[:])
            for g in range(2):
                gsl = slice(g * 512, (g + 1) * 512)
                tsl = slice(4 * g, 4 * g + 4)
                ath = pc_ps.tile([D, 512], f32, tag="pc", name=f"at{b}_{g}")
                nc.tensor.matmul(ath[:], qt_sb[:], e1t_sb[:, gsl],
                                 start=True, stop=True)
                bmh = pc_ps.tile([D, 512], f32, tag="pc", name=f"bm{b}_{g}")
                nc.tensor.matmul(bmh[:], that_sb[:], e1t_sb[:, gsl],
                                 start=True, stop=True)
                athp = ath[:].rearrange("d (t c) -> d c t", c=D)[:, 0:TL, :]
                bmhp = bmh[:].rearrange("d (t c) -> d c t", c=D)[:, 0:TL, :]
                s1bp = s1bb[:, gsl].rearrange(
                    "d (t c) -> d c t", c=D)[:, 0:TL, :]
                oav = oa[:].rearrange("d (p t) -> d p t", t=NT)[:, :, tsl]
                ocav = oca[:].rearrange("d (p t) -> d p t", t=NT)[:, :, tsl]
                ocbv = ocb[:].rearrange("d (p t) -> d p t", t=NT)[:, :, tsl]
                nc.vector.tensor_tensor(out=oav, in0=athp, in1=s1bp,
                                        op=ALU.mult)
                nc.vector.tensor_tensor(out=ocav, in0=athp,
                                        in1=cs1v[:, :, tsl], op=ALU.mult)
                nc.vector.tensor_tensor(out=ocbv, in0=bmhp,
                                        in1=cs1v[:, :, tsl], op=ALU.mult)
            nc.sync.dma_start(O_d[b, D:2 * D], oa[:])
            nc.sync.dma_start(O_d[b, 2 * D:3 * D], oca[:])
            nc.sync.dma_start(O_d[b, 3 * D:4 * D], ocb[:])

        # software-pipelined emission: A(b) | B(b-1) | C(b-2)
        for step in range(NB + 2):
            if step < NB:
                stage_a(step)
            if 1 <= step <= NB:
                stage_b(step - 1)
            if step >= 2:
                stage_c(step - 2)

    nc.compile()
    return nc


def _get_nc(**kw):
    key = tuple(sorted(kw.items()))
    if key not in _cache:
        _cache[key] = _build(**kw)
    return _cache[key]


def kernel(C, Q, W, **build_kw):
    from concourse import bass_utils

    C = np.ascontiguousarray(C, np.float32)
    Q = np.ascontiguousarray(Q, np.float32)
    Wr = np.ascontiguousarray(W, np.float32).reshape(NCORES, NB, LC, 3 * D)
    Cs = C.reshape(NCORES, NB, D, LC)
    Qs = Q.reshape(NCORES, NB, D, LQ)

    nc = _get_nc(**build_kw)
    in_maps = [{"C": Cs[i], "Q": Qs[i], "W": Wr[i]} for i in range(NCORES)]
    res = bass_utils.run_bass_kernel_spmd(nc, in_maps,
                                          core_ids=list(range(NCORES)))
    out = np.concatenate([res.results[i]["OUT"] for i in range(NCORES)], 0)
    return out.astype(np.float32)


# revision 29
# speedup vs baseline: 1.5923x; 1.2136x over previous
"""CQAttention (context-query attention) Bass kernel for 8 NeuronCores.

Full inputs:  C [64,128,1000] f32, Q [64,128,100] f32, W [64000,1,384] f32
Full output:  [64, 512, 1000] f32

Sharding: pure data-parallel on the batch dim - 8 batches per core.

Per-batch math (D=128, Lc=1000, Lq=100):
  Ct = C.T [Lc,D], Qt = Q.T [Lq,D], w1/w2/w3 = W row blocks [Lc,D]
  U  = w1 + w3*Ct ; v = rowsum(w2*Ct)
  S  = U @ Q + v  (the v term drops out of the row softmax S1)
  S1 = softmax_cols(S) ; S2 = softmax_rows(S)
  A  = S1 @ Qt ; Bm = S1 @ (S2^T @ Ct)
  out = concat([Ct, A, Ct*A, Ct*Bm], 1).T  -> [4D, Lc]

Layout notes:
 - Lc is tiled 8 x 125 with the INTERLEAVED mapping i = p*8 + t (p =
   partition, t = tile) so the W DMA reads 12KB contiguous per partition.
   All intermediate tensors with an Lc axis are kept in the permuted
   (t-major) order; the final output ops unpermute via strided APs.
 - Scores are built transposed (S0T [Lq, Lc]) so the big matmuls run with
   float32r operands at full PE rate (moving free dim >= 256).
 - S1 normalization: column sums of exp(S0T) via a ones-vector matmul,
   reciprocal, then a K=1 matmul broadcast across partitions.
 - S2 path: PE-transpose exp(S0T) tiles, scale by exp(v), cast bf16, and
   contract with bf16 Ct tiles (ones column appended for the s2 sums).
"""

import numpy as np

B, D, LC, LQ = 64, 128, 1000, 100
NCORES = 8
NB = B // NCORES   # batches per core
NT = 8             # LC tiles
TL = LC // NT      # 125

_cache = {}


def _build(tu_bf16=True, v_bf16=True):
    import concourse.bass as bass
    import concourse.tile as tile
    from concourse import bacc, mybir, masks
    from contextlib import ExitStack

    f32 = mybir.dt.float32
    f32r = mybir.dt.float32r
    bf16 = mybir.dt.bfloat16
    AF = mybir.ActivationFunctionType
    ALU = mybir.AluOpType
    AX = mybir.AxisListType

    ct_dt = bf16 if tu_bf16 else f32
    e2_dt = bf16 if tu_bf16 else f32

    nc = bacc.Bacc("TRN2", target_bir_lowering=False, debug=False,
                   num_devices=NCORES)
    C_d = nc.dram_tensor("C", [NB, D, LC], f32, kind="ExternalInput").ap()
    Q_d = nc.dram_tensor("Q", [NB, D, LQ], f32, kind="ExternalInput").ap()
    W_d = nc.dram_tensor("W", [NB, LC, 3 * D], f32, kind="ExternalInput").ap()
    O_d = nc.dram_tensor("OUT", [NB, 4 * D, LC], f32, kind="ExternalOutput").ap()

    with tile.TileContext(nc) as tc, ExitStack() as ctx:
        const_pool = ctx.enter_context(tc.tile_pool(name="const", bufs=1))
        ident = const_pool.tile([128, 128], f32)
        masks.make_identity(nc, ident[:])
        identr = const_pool.tile([128, 128], f32r)
        nc.scalar.activation(identr[:], ident[:], AF.Copy)
        # f32r ones for the s1 column-sum / broadcast matmuls
        ones_f = const_pool.tile([128, 1], f32)
        nc.vector.memset(ones_f[:], 1.0)
        ones_col = const_pool.tile([128, 1], f32r)
        nc.scalar.activation(ones_col[:], ones_f[:], AF.Copy)
        ones_rf = const_pool.tile([1, 128], f32)
        nc.vector.memset(ones_rf[:], 1.0)
        ones_row = const_pool.tile([1, 128], f32r)
        nc.scalar.activation(ones_row[:], ones_rf[:], AF.Copy)
        zero_f = const_pool.tile([128, 1], f32)
        nc.vector.memset(zero_f[:], 0.0)

        sb = ctx.enter_context(tc.tile_pool(name="sb", bufs=2))
        small = ctx.enter_context(tc.tile_pool(name="small", bufs=3))
        outp = ctx.enter_context(tc.tile_pool(name="outp", bufs=2))
        tp_ps = ctx.enter_context(tc.tile_pool(name="tp_ps", bufs=3, space="PSUM"))
        mm_ps = ctx.enter_context(tc.tile_pool(name="mm_ps", bufs=5, space="PSUM"))

        for b in range(NB):
            # ---- loads ----
            # w_sb[p, t, c] = W[b, p*8+t, c]  (12KB contiguous per partition)
            w_sb = sb.tile([TL, NT * 3 * D], f32, tag="w", name=f"w{b}")
            nc.gpsimd.dma_start(
                w_sb[:].rearrange("p (t c) -> p t c", c=3 * D),
                W_d[b].rearrange("(p t) c -> p t c", t=NT))
            c_sb = sb.tile([D, LC], f32, tag="c", name=f"c{b}")
            nc.sync.dma_start(c_sb[:], C_d[b])
            q_sb = sb.tile([D, LQ], f32, tag="q", name=f"q{b}")
            nc.sync.dma_start(q_sb[:], Q_d[b])

            # ---- Qt (early: only needs the Q load) ----
            qtp = tp_ps.tile([LQ, D], f32, tag="tp", name=f"qtp{b}")
            nc.tensor.transpose(qtp[:], q_sb[:], ident[:])
            qt_sb = small.tile([LQ, D], f32r, tag="qt", name=f"qt{b}")
            nc.scalar.activation(qt_sb[:], qtp[:], AF.Copy)
            q_r = small.tile([D, LQ], f32r, tag="q_r", name=f"q_r{b}")
            nc.scalar.activation(q_r[:], q_sb[:], AF.Copy)

            # views with the interleaved Lc mapping  i = p*8 + t
            c_tiles = c_sb[:].rearrange("d (p t) -> d t p", t=NT)  # [D, t, p]
            wv = w_sb[:].rearrange("p (t c) -> p t c", c=3 * D)
            w1 = wv[:, :, 0:D]
            w2 = wv[:, :, D:2 * D]
            w3 = wv[:, :, 2 * D:3 * D]

            # ---- Ct tiles: PE transpose groups of 4; keep PSUM f32 copy
            #      for U/v, write bf16 (+ones col) SBUF copy for Tu ----
            ct_sb = sb.tile([TL, NT * (D + 1)], ct_dt, tag="ct", name=f"ct{b}")
            ctv = ct_sb[:].rearrange("p (t c) -> p t c", c=D + 1)
            nc.vector.memset(ctv[:, :, D:D + 1], 1.0)
            w3ct = sb.tile([TL, NT * D], f32, tag="w3ct", name=f"w3ct{b}")
            w3ctv = w3ct[:].rearrange("p (t c) -> p t c", c=D)
            vtmp = sb.tile([TL, NT * D], f32, tag="vtmp", name=f"vtmp{b}")
            vtmpv = vtmp[:].rearrange("p (t c) -> p t c", c=D)
            for g in range(2):
                ctp = tp_ps.tile([TL, 4 * D], f32, tag="tp", name=f"ctp{b}_{g}")
                for k in range(4):
                    t = 4 * g + k
                    nc.tensor.transpose(
                        ctp[:, k * D:(k + 1) * D], c_tiles[:, t, :], ident[:])
                ctpv = ctp[:].rearrange("p (k c) -> p k c", c=D)
                gs = slice(4 * g, 4 * g + 4)
                nc.scalar.activation(ctv[:, gs, 0:D], ctpv, AF.Copy)
                nc.vector.tensor_tensor(out=w3ctv[:, gs, :], in0=w3[:, gs, :],
                                        in1=ctpv, op=ALU.mult)
                if not v_bf16:
                    nc.vector.tensor_tensor(out=vtmpv[:, gs, :],
                                            in0=w2[:, gs, :], in1=ctpv,
                                            op=ALU.mult)
            if v_bf16:
                # gpsimd is idle; feed it the v multiply from the bf16 ct
                nc.gpsimd.tensor_tensor(out=vtmpv, in0=w2,
                                        in1=ctv[:, :, 0:D], op=ALU.mult)
            v_all = small.tile([TL, NT], f32, tag="v", name=f"v{b}")
            nc.vector.tensor_reduce(v_all[:], vtmpv, axis=AX.X, op=ALU.add)
            expv = small.tile([TL, NT], f32, tag="expv", name=f"expv{b}")
            nc.scalar.activation(expv[:], v_all[:], AF.Exp)

            # ---- U^T via PSUM-accumulating PE transposes:
            #      utp = w1_tile^T  (+)  (w3*Ct)_tile^T  ----
            ut_sb = sb.tile([D, 8 * D], f32r, tag="ut", name=f"ut{b}")
            utv = ut_sb[:].rearrange("d (t c) -> d t c", c=D)
            nc.scalar.activation(
                utv[:, :, TL:D],
                zero_f[:, 0:1].to_broadcast((D, NT, D - TL)), AF.Copy)
            e1t_sb = sb.tile([LQ, 8 * D], f32r, tag="e1t", name=f"e1t{b}")
            s0h = []
            for g in range(2):
                utp = tp_ps.tile([D, 4 * TL], f32, tag="tp", name=f"utp{b}_{g}")
                for k in range(4):
                    t = 4 * g + k
                    nc.tensor.matmul(utp[:, k * TL:(k + 1) * TL],
                                     w_sb[:, t * 3 * D:t * 3 * D + D],
                                     ident[0:TL, 0:TL],
                                     is_transpose=True, start=True, stop=False)
                    nc.tensor.matmul(utp[:, k * TL:(k + 1) * TL],
                                     w3ct[:, t * D:(t + 1) * D],
                                     ident[0:TL, 0:TL],
                                     is_transpose=True, start=False, stop=True)
                nc.scalar.activation(
                    utv[:, 4 * g:4 * g + 4, 0:TL],
                    utp[:].rearrange("d (k c) -> d k c", c=TL), AF.Copy)
                s0g = mm_ps.tile([LQ, 512], f32, tag="mmh", name=f"s0t{b}_{g}")
                s0h.append(s0g)
                nc.tensor.matmul(s0g[:], q_r[:],
                                 ut_sb[:, g * 512:(g + 1) * 512],
                                 start=True, stop=True)
                nc.scalar.activation(e1t_sb[:, g * 512:(g + 1) * 512],
                                     s0g[:], AF.Exp)

            # ---- s1 normalization: 1/colsum(E1T) = exp(-ln(sum)), then a
            #      K=1 matmul broadcast over the Lq partitions ----
            s1t = sb.tile([LQ, 8 * D], f32r, tag="s1t", name=f"s1t{b}")
            s1ln = small.tile([1, 8 * D], f32, tag="s1ln", name=f"s1ln{b}")
            s1ri = small.tile([1, 8 * D], f32r, tag="s1ri", name=f"s1ri{b}")
            for g in range(2):
                ssum = tp_ps.tile([1, 512], f32, tag="tp", name=f"ssum{b}_{g}")
                nc.tensor.matmul(ssum[:], ones_col[0:LQ, :],
                                 e1t_sb[:, g * 512:(g + 1) * 512],
                                 start=True, stop=True)
                gsl = slice(g * 512, (g + 1) * 512)
                nc.scalar.activation(s1ln[:, gsl], ssum[:], AF.Ln)
                nc.scalar.activation(s1ri[:, gsl], s1ln[:, gsl], AF.Exp,
                                     scale=-1.0)
                bch = mm_ps.tile([LQ, 512], f32, tag="mmh", name=f"bch{b}_{g}")
                nc.tensor.matmul(bch[:], ones_row[:, 0:LQ], s1ri[:, gsl],
                                 start=True, stop=True)
                nc.vector.tensor_tensor(out=s1t[:, gsl],
                                        in0=e1t_sb[:, gsl], in1=bch[:],
                                        op=ALU.mult)

            # ---- E2 tiles = transpose(E1T) * exp(v) ----
            e2_all = sb.tile([TL, NT * LQ], e2_dt, tag="e2", name=f"e2{b}")
            e2v = e2_all[:].rearrange("p (t c) -> p t c", c=LQ)
            for g in range(2):
                e1p = tp_ps.tile([TL, 4 * LQ], f32r, tag="tp",
                                 name=f"e1p{b}_{g}")
                for k in range(4):
                    t = 4 * g + k
                    nc.tensor.transpose(
                        e1p[:, k * LQ:(k + 1) * LQ],
                        e1t_sb[:, t * D:t * D + TL],
                        identr[0:LQ, 0:LQ])
                e1pv = e1p[:].rearrange("p (k c) -> p k c", c=LQ)
                scl = expv[:, 4 * g:4 * g + 4].unsqueeze(-1).to_broadcast(
                    (TL, 4, LQ))
                nc.vector.tensor_tensor(
                    out=e2v[:, 4 * g:4 * g + 4, :], in0=e1pv,
                    in1=scl, op=ALU.mult)

            # ---- Tu = E2^T @ [Ct | 1]  (accumulate over tiles) ----
            tu = tp_ps.tile([LQ, D + 1], f32, tag="tp", name=f"tu{b}")
            for t in range(NT):
                nc.tensor.matmul(tu[:], e2v[:, t, :], ctv[:, t, :],
                                 start=(t == 0), stop=(t == NT - 1))
            s2r = small.tile([LQ, 1], f32, tag="s2r", name=f"s2r{b}")
            nc.vector.reciprocal(s2r[:], tu[:, D:D + 1])
            that_sb = small.tile([LQ, D], f32r, tag="that", name=f"that{b}")
            nc.vector.tensor_scalar_mul(that_sb[:], tu[:, 0:D], s2r[:])

            # ---- A^T and Bm^T (per half) + unpermuted outputs ----
            cpt = c_sb[:].rearrange("d (p t) -> d p t", t=NT)
            oa = outp.tile([D, LC], f32, tag="oa", name=f"oa{b}")
            oca = outp.tile([D, LC], f32, tag="oca", name=f"oca{b}")
            ocb = outp.tile([D, LC], f32, tag="ocb", name=f"ocb{b}")
            nc.sync.dma_start(O_d[b, 0:D], c_sb[:])
            for g in range(2):
                gsl = slice(g * 512, (g + 1) * 512)
                tsl = slice(4 * g, 4 * g + 4)
                ath = mm_ps.tile([D, 512], f32, tag="mmh", name=f"at{b}_{g}")
                nc.tensor.matmul(ath[:], qt_sb[:], s1t[:, gsl],
                                 start=True, stop=True)
                bmh = mm_ps.tile([D, 512], f32, tag="mmh", name=f"bm{b}_{g}")
                nc.tensor.matmul(bmh[:], that_sb[:], s1t[:, gsl],
                                 start=True, stop=True)
                athp = ath[:].rearrange("d (t c) -> d c t", c=D)[:, 0:TL, :]
                bmhp = bmh[:].rearrange("d (t c) -> d c t", c=D)[:, 0:TL, :]
                oav = oa[:].rearrange("d (p t) -> d p t", t=NT)[:, :, tsl]
                ocav = oca[:].rearrange("d (p t) -> d p t", t=NT)[:, :, tsl]
                ocbv = ocb[:].rearrange("d (p t) -> d p t", t=NT)[:, :, tsl]
                cpg = cpt[:, :, tsl]
                nc.scalar.activation(oav, athp, AF.Copy)
                nc.vector.tensor_tensor(out=ocav, in0=cpg, in1=athp,
                                        op=ALU.mult)
                nc.vector.tensor_tensor(out=ocbv, in0=cpg, in1=bmhp,
                                        op=ALU.mult)
            nc.sync.dma_start(O_d[b, D:2 * D], oa[:])
            nc.sync.dma_start(O_d[b, 2 * D:3 * D], oca[:])
            nc.sync.dma_start(O_d[b, 3 * D:4 * D], ocb[:])

    nc.compile()
    return nc


def _get_nc(**kw):
    key = tuple(sorted(kw.items()))
    if key not in _cache:
        _cache[key] = _build(**kw)
    return _cache[key]


def kernel(C, Q, W, **build_kw):
    from concourse import bass_utils

    C = np.ascontiguousarray(C, np.float32)
    Q = np.ascontiguousarray(Q, np.float32)
    Wr = np.ascontiguousarray(W, np.float32).reshape(NCORES, NB, LC, 3 * D)
    Cs = C.reshape(NCORES, NB, D, LC)
    Qs = Q.reshape(NCORES, NB, D, LQ)

    nc = _get_nc(**build_kw)
    in_maps = [{"C": Cs[i], "Q": Qs[i], "W": Wr[i]} for i in range(NCORES)]
    res = bass_utils.run_bass_kernel_spmd(nc, in_maps,
                                          core_ids=list(range(NCORES)))
    out = np.concatenate([res.results[i]["OUT"] for i in range(NCORES)], 0)
    return out.astype(np.float32)
